# revision 17
# baseline (speedup 1.0000x reference)
"""Trainium2 Bass kernel for nn_AdaBP_Decoder (LDPC adaptive BP, T=30 iters).

Strategy: shard edges/variables/checks across 8 NeuronCores; batch B=128 lives
on the 128 SBUF partitions. Per core: col-domain tensors (128, 1536) hold its
1536 edges sorted by variable (seg3 = strided free reduce); row-domain tensors
hold the 1536 edges of its checks sorted by check (seg6). The two per-iteration
edge reorders (col->row of lam, row->col of C~=We*msg_C2V) go: gpsimd
local_scatter into dest-core-grouped fp16 blocks -> AllGather -> static block
transpose -> ReduceScatter (the two collectives supply all rank awareness, so
no dynamic access patterns are needed; the x8 sum of identical copies is exact
in fp16 and folded into downstream scales) -> local_scatter into domain order.
Check-node update uses the signed
tanh-product/divide form (atanh via two Ln's); damping is done as diagonal
matmuls accumulated in PSUM on the TensorEngine.
"""
import sys
sys.path.insert(0, "/opt/trn_rl_repo")
import numpy as np

N, M, COL_DEG, E, B, T = 4096, 2048, 3, 12288, 128, 30
NC = 8
NL, ML, EL = N // NC, M // NC, E // NC
P = 128

LLR_CLIP = 15.0
UCLIP = float(np.tanh(np.float32(15.0) / 2.0))
EPS1 = 1.0 - 1e-6
LOG10SCALE = 10.0 / float(np.log(10.0))

_cache = {}


def _build_tables(row_idx, col_idx):
    row = np.asarray(row_idx).astype(np.int64)
    col = np.asarray(col_idx).astype(np.int64)
    assert row.shape == (E,) and col.shape == (E,)
    assert np.all(np.bincount(col, minlength=N) == COL_DEG)
    assert np.all(np.bincount(row, minlength=M) == E // M)
    colcore = col // NL
    rowcore = row // ML

    localcol, localrow = [], []
    for c in range(NC):
        eids = np.where(colcore == c)[0]
        localcol.append(eids[np.lexsort((eids, col[eids]))])
    for d in range(NC):
        eids = np.where(rowcore == d)[0]
        localrow.append(eids[np.lexsort((eids, row[eids]))])
    poscol = np.empty(E, np.int64)
    posrow = np.empty(E, np.int64)
    for c in range(NC):
        poscol[localcol[c]] = np.arange(EL)
        posrow[localrow[c]] = np.arange(EL)

    S = [[None] * NC for _ in range(NC)]    # col->row blocks
    Tb = [[None] * NC for _ in range(NC)]   # row->col blocks
    for c in range(NC):
        eids = localcol[c]
        for d in range(NC):
            sel = np.where(rowcore[eids] == d)[0]
            S[c][d] = sel[np.argsort(posrow[eids[sel]])]
    for d in range(NC):
        eids = localrow[d]
        for c in range(NC):
            sel = np.where(colcore[eids] == c)[0]
            Tb[d][c] = sel[np.argsort(poscol[eids[sel]])]

    maxblk = max(len(S[c][d]) for c in range(NC) for d in range(NC))
    PAD = maxblk + (maxblk & 1)
    assert NC * PAD * 32 < (1 << 16), f"PAD={PAD} too large for local_scatter"

    # scatter tables (per core), int16:
    #   ssc[c][i]  : col-pos i -> slot in send buffer (d*PAD + k)       [c2r send]
    #   rsc[d][s]  : send-slot s (8*PAD) -> row-pos (or -1 pad)          [c2r recv]
    #   ssr[d][i]  : row-pos i -> slot (c*PAD + k)                       [r2c send]
    #   rsr[c][s]  : slot s -> col-pos (or -1)                           [r2c recv]
    ssc = np.zeros((NC, EL), np.int16)
    rsc = np.full((NC, NC * PAD), -1, np.int16)
    ssr = np.zeros((NC, EL), np.int16)
    rsr = np.full((NC, NC * PAD), -1, np.int16)
    for c in range(NC):
        for d in range(NC):
            blk = S[c][d]
            ssc[c, blk] = (d * PAD + np.arange(len(blk))).astype(np.int16)
    for d in range(NC):
        for c in range(NC):
            blk = S[c][d]                      # positions in localcol[c]
            gids = localcol[c][blk]
            rsc[d, c * PAD: c * PAD + len(blk)] = posrow[gids].astype(np.int16)
    for d in range(NC):
        for c in range(NC):
            blk = Tb[d][c]
            ssr[d, blk] = (c * PAD + np.arange(len(blk))).astype(np.int16)
    for c in range(NC):
        for d in range(NC):
            blk = Tb[d][c]
            gids = localrow[d][blk]
            rsr[c, d * PAD: d * PAD + len(blk)] = poscol[gids].astype(np.int16)

    return dict(PAD=PAD, ssc=ssc, rsc=rsc, ssr=ssr, rsr=rsr)


def _build_program(PAD, T_run=T):
    import concourse.bass as bass
    import concourse.bacc as bacc
    import concourse.mybir as mybir
    from concourse.tile import TileContext

    dt, op = mybir.dt, mybir.AluOpType
    AF = mybir.ActivationFunctionType
    AX = mybir.AxisListType
    SP = NC * PAD   # send buffer width

    nc = bacc.Bacc(None, target_bir_lowering=False)

    chn_ext = nc.declare_dram_parameter("chn", [N, B], dt.float32, isOutput=False)
    chm_ext = nc.declare_dram_parameter("chm", [NL, B], dt.float32, isOutput=False)
    idn_ext = nc.declare_dram_parameter("idn", [P, P], dt.float32, isOutput=False)
    w1_ext = nc.declare_dram_parameter("w1s", [3, 1, 20], dt.float32, isOutput=False)
    b1_ext = nc.declare_dram_parameter("b1s", [3, 20, 1], dt.float32, isOutput=False)
    w2_ext = nc.declare_dram_parameter("w2s", [3, 20, 1], dt.float32, isOutput=False)
    b2_ext = nc.declare_dram_parameter("b2s", [3, 1, 1], dt.float32, isOutput=False)
    ssc_ext = nc.declare_dram_parameter("ssc", [P, EL], dt.int16, isOutput=False)
    rsc_ext = nc.declare_dram_parameter("rsc", [P, SP], dt.int16, isOutput=False)
    ssr_ext = nc.declare_dram_parameter("ssr", [P, EL], dt.int16, isOutput=False)
    rsr_ext = nc.declare_dram_parameter("rsr", [P, SP], dt.int16, isOutput=False)
    out_ext = nc.declare_dram_parameter("out", [T_run, NL, B], dt.float16, isOutput=True)

    aginL = nc.dram_tensor("aginL", [P, SP], dt.float16)
    agoutL = nc.dram_tensor("agoutL", [NC, P, SP], dt.float16, addr_space="Shared")
    rshL = nc.dram_tensor("rshL", [NC, P, SP], dt.float16)
    rsoL = nc.dram_tensor("rsoL", [P, SP], dt.float16)
    aginR = nc.dram_tensor("aginR", [P, SP], dt.float16)
    agoutR = nc.dram_tensor("agoutR", [NC, P, SP], dt.float16, addr_space="Shared")
    rshR = nc.dram_tensor("rshR", [NC, P, SP], dt.float16)
    rsoR = nc.dram_tensor("rsoR", [P, SP], dt.float16)

    with TileContext(nc) as tc:
        with (
            tc.tile_pool(name="sbuf", bufs=1) as pool,
            tc.tile_pool(name="psum", bufs=1, space="PSUM") as pp,
        ):
            f32, f16, i16 = dt.float32, dt.float16, dt.int16
            # --- persistent tiles
            V = pool.tile([P, EL], f32)
            Ccol = pool.tile([P, EL], f16)
            Crow = pool.tile([P, EL], f16)
            ell = pool.tile([P, NL], f32)
            ident = pool.tile([P, P], f32)
            issc = pool.tile([P, EL], i16)
            irsc = pool.tile([P, SP], i16)
            issr = pool.tile([P, EL], i16)
            irsr = pool.tile([P, SP], i16)
            dOm = pool.tile([P, P], f32)
            dG = pool.tile([P, P], f32)
            dGn = pool.tile([P, P], f32)
            dGWe = pool.tile([P, P], f32)
            dGWen = pool.tile([P, P], f32)
            dOm16 = pool.tile([P, P], f16)
            dGn16 = pool.tile([P, P], f16)
            # working tiles
            lam16 = pool.tile([P, EL], f16)
            lamR = pool.tile([P, EL], f16)
            sbA = pool.tile([P, SP], f16)
            sbB = pool.tile([P, SP], f16)
            u0t = pool.tile([P, EL], f32)
            ut = pool.tile([P, EL], f32)
            Prt = pool.tile([P, ML], f32)
            rct = pool.tile([P, EL], f32)
            wct = pool.tile([P, EL], f32)
            t1t = pool.tile([P, EL], f32)
            t2t = pool.tile([P, EL], f32)
            s3t = pool.tile([P, NL], f32)
            tts = [pool.tile([P, NL], f32, name=f"tt{j}") for j in range(2)]
            touts = [pool.tile([P, NL], f16, name=f"tout{j}") for j in range(2)]
            # psum tiles: 4x384 (V-damp), 3x512 (C-damp), 1x512 (transposes/misc)
            psV = [pp.tile([P, 384], f32, name=f"psV{j}", tag=f"psV{j}", space="PSUM") for j in range(4)]
            psC = [pp.tile([P, 512], f32, name=f"psC{j}", tag=f"psC{j}", space="PSUM") for j in range(3)]
            psT = pp.tile([P, 512], f32, space="PSUM", tag="psT")

            # --- load inputs
            chn_nat = pool.tile([P, N], f32, tag="bigA")
            nc.scalar.dma_start(out=chn_nat[:].rearrange("p (k b) -> p k b", b=B), in_=chn_ext[:].rearrange("(k p) b -> p k b", p=P))
            chm_nat = pool.tile([P, 4 * P], f32)
            nc.scalar.dma_start(out=chm_nat[:].rearrange("p (k b) -> p k b", b=B), in_=chm_ext[:].rearrange("(k p) b -> p k b", p=P))
            nc.scalar.dma_start(out=ident[:], in_=idn_ext[:])
            nc.scalar.dma_start(out=issc[:], in_=ssc_ext[:])
            nc.scalar.dma_start(out=irsc[:], in_=rsc_ext[:])
            nc.scalar.dma_start(out=issr[:], in_=ssr_ext[:])
            nc.scalar.dma_start(out=irsr[:], in_=rsr_ext[:])
            w1t = pool.tile([1, 60], f32)  # lhsT layouts (3 nets x 20, 1 partition)
            b1t = pool.tile([20, 3], f32)
            w2t = pool.tile([20, 3], f32)
            b2t = pool.tile([1, 3], f32)
            nc.scalar.dma_start(out=w1t[:].rearrange("a (s j) -> a s j", s=3), in_=w1_ext[:].rearrange("s a j -> a s j"))
            nc.scalar.dma_start(out=b1t[:], in_=b1_ext[:].rearrange("s j a -> j s a"))
            nc.scalar.dma_start(out=w2t[:], in_=w2_ext[:].rearrange("s j a -> j s a"))
            nc.scalar.dma_start(out=b2t[:], in_=b2_ext[:].rearrange("s j a -> j s a"))

            # --- adapter nets: Eng -> snr(ln q) -> 3 tiny MLPs -> gamma/Wi/We
            sq = pool.tile([P, N], f32, tag="bigB")
            nc.scalar.activation(out=sq[:], in_=chn_nat[:], func=AF.Square, scale=1.0 / 64.0)
            part = pool.tile([P, P], f32)
            nc.vector.tensor_reduce(out=part[:], in_=sq[:].rearrange("p (k b) -> p b k", k=N // P),
                                    axis=AX.X, op=op.add)
            ones_col = pool.tile([P, 1], f32)
            nc.vector.memset(ones_col[:], 1.0)
            psE = pp.tile([1, P], f32, space="PSUM", tag="psC0")
            nc.tensor.matmul(out=psE[:], lhsT=ones_col[:], rhs=part[:], start=True, stop=True)
            Eng = pool.tile([1, P], f32)
            nc.vector.tensor_copy(Eng[:], psE[:])
            s1 = pool.tile([1, P], f32)
            nc.scalar.activation(out=s1[:], in_=Eng[:], func=AF.Sqrt, bias=1.0, scale=1.0)
            dn = pool.tile([1, P], f32)
            nc.vector.tensor_scalar(out=dn[:], in0=s1[:], scalar1=1.0, scalar2=2.0,
                                    op0=op.add, op1=op.mult)
            rdn = pool.tile([1, P], f32)
            nc.vector.reciprocal(out=rdn[:], in_=dn[:])
            qq = pool.tile([1, P], f32)
            nc.vector.scalar_tensor_tensor(out=qq[:], in0=Eng[:], scalar=1.0, in1=rdn[:],
                                           op0=op.mult, op1=op.mult)
            lnq = pool.tile([1, P], f32)
            nc.scalar.activation(out=lnq[:], in_=qq[:], func=AF.Ln)

            rows = [pool.tile([1, P], f32, name=f"rows{s}") for s in range(3)]
            psH = pp.tile([20, P], f32, space="PSUM", tag="psC1")
            psO = pp.tile([1, P], f32, space="PSUM", tag="psC2")
            hX = pool.tile([20, P], f32)
            for s in range(3):
                nc.tensor.matmul(out=psH[:], lhsT=w1t[:, 20 * s:20 * (s + 1)], rhs=lnq[:],
                                 start=True, stop=True)
                nc.scalar.activation(out=hX[:], in_=psH[:], func=AF.Relu, bias=b1t[:, s:s + 1])
                nc.tensor.matmul(out=psO[:], lhsT=w2t[:, s:s + 1], rhs=hX[:], start=True, stop=True)
                nc.scalar.activation(out=rows[s][:], in_=psO[:], func=AF.Sigmoid,
                                     bias=b2t[:, s:s + 1])
            # transpose rows -> per-partition columns
            ones11 = pool.tile([1, 1], f32)
            nc.vector.memset(ones11[:], 1.0)
            psPP = pp.tile([P, 3], f32, space="PSUM", tag="psT")
            gpp = pool.tile([P, 1], f32)
            wipp = pool.tile([P, 1], f32)
            wepp = pool.tile([P, 1], f32)
            for s, dst in enumerate([gpp, wipp, wepp]):
                nc.tensor.matmul(out=psPP[:, s:s + 1], lhsT=rows[s][:], rhs=ones11[:],
                                 start=True, stop=True)
                nc.vector.tensor_copy(dst[:], psPP[:, s:s + 1])
            ompp = pool.tile([P, 1], f32)
            nc.vector.tensor_scalar(out=ompp[:], in0=gpp[:], scalar1=-1.0, scalar2=1.0,
                                    op0=op.mult, op1=op.add)
            gwepp = pool.tile([P, 1], f32)
            nc.vector.tensor_tensor(gwepp[:], gpp[:], wepp[:], op.mult)
            gwenpp = pool.tile([P, 1], f32)
            nc.vector.tensor_scalar_mul(out=gwenpp[:], in0=gwepp[:], scalar1=-1.0)
            gnpp = pool.tile([P, 1], f32)
            nc.vector.tensor_scalar_mul(out=gnpp[:], in0=gpp[:], scalar1=-0.125)
            # diag matrices
            nc.vector.tensor_scalar_mul(out=dOm[:], in0=ident[:], scalar1=ompp[:])
            nc.vector.tensor_scalar_mul(out=dG[:], in0=ident[:], scalar1=gpp[:])
            nc.vector.tensor_scalar_mul(out=dGn[:], in0=ident[:], scalar1=gnpp[:])
            nc.vector.tensor_scalar_mul(out=dGWe[:], in0=ident[:], scalar1=gwepp[:])
            nc.vector.tensor_scalar_mul(out=dGWen[:], in0=ident[:], scalar1=gwenpp[:])
            nc.vector.tensor_copy(dOm16[:], dOm[:])
            nc.vector.tensor_copy(dGn16[:], dGn[:])
            # ell = Wi * chn_mine^T
            for k in range(4):
                nc.tensor.transpose(out=psT[:, :P], in_=chm_nat[:, k * P:(k + 1) * P],
                                    identity=ident[:])
                nc.vector.tensor_scalar_mul(out=ell[:, k * P:(k + 1) * P], in0=psT[:, :P],
                                            scalar1=wipp[:])
            # init state
            nc.vector.memset(V[:], 0.0)
            nc.vector.memset(Ccol[:], 0.0)
            nc.vector.memset(Crow[:], 0.0)

            # --- helper emitters
            def emit_t(i):
                t = tts[i % 2]
                nc.vector.tensor_reduce(out=s3t[:], in_=Ccol[:].rearrange("p (v j) -> p v j", j=3),
                                        axis=AX.X, op=op.add)
                nc.vector.scalar_tensor_tensor(out=t[:], in0=s3t[:], scalar=0.125, in1=ell[:], op0=op.mult, op1=op.add)
                return t

            def emit_out(i, t):
                tout = touts[i % 2]
                for k in range(4):
                    nc.tensor.transpose(out=psT[:, :P],
                                        in_=t[:, k * P:(k + 1) * P], identity=ident[:])
                    nc.vector.tensor_copy(tout[:, k * P:(k + 1) * P], psT[:, :P])
                nc.scalar.dma_start(
                    out=out_ext[i].rearrange("(k nl) b -> nl k b", k=4),
                    in_=tout[:].rearrange("p (k b) -> p k b", k=4))

            def transport(src16, sidx, agin, agout, rsh, rso, ridx, dst16):
                nc.gpsimd.local_scatter(out_ap=sbA[:], data_ap=src16[:], idxs_ap=sidx[:],
                                        channels=P, num_elems=SP, num_idxs=EL)
                nc.scalar.dma_start(out=agin[:], in_=sbA[:])
                nc.gpsimd.collective_compute(
                    "AllGather", op.bypass, replica_groups=[list(range(NC))],
                    ins=[agin[:]], outs=[agout[:]])
                # static block transpose: rsh[d, p, c*PAD:...] = agout[c, p, d*PAD:...]
                for c_ in range(NC):
                    nc.scalar.dma_start(
                        out=rsh[:, :, c_ * PAD:(c_ + 1) * PAD],
                        in_=agout[c_].rearrange("p (d f) -> d p f", d=NC))
                nc.gpsimd.collective_compute(
                    "ReduceScatter", op.add, replica_groups=[list(range(NC))],
                    ins=[rsh[:]], outs=[rso[:]])
                nc.scalar.dma_start(out=sbB[:], in_=rso[:])
                nc.gpsimd.local_scatter(out_ap=dst16[:], data_ap=sbB[:], idxs_ap=ridx[:],
                                        channels=P, num_elems=EL, num_idxs=SP)

            # --- main loop (fully unrolled)
            for i in range(T_run):
                t = emit_t(i)
                if i >= 1:
                    emit_out(i - 1, t)
                # V-damp: V' = (1-g)V + g*t[rep3] - g*Ccol ; lam = clip(V')
                for j in range(4):
                    sl = slice(384 * j, 384 * (j + 1))
                    nc.tensor.matmul(out=psV[j][:], lhsT=dOm[:], rhs=V[:, sl],
                                     start=True, stop=False)
                    nc.tensor.matmul(out=psV[j][:], lhsT=dG[:],
                                     rhs=t[:, 128 * j:128 * (j + 1)].unsqueeze(2)
                                     .broadcast_to([P, 128, 3]),
                                     start=False, stop=False)
                    nc.tensor.matmul(out=psV[j][:], lhsT=dGn16[:], rhs=Ccol[:, sl],
                                     start=False, stop=True)
                    nc.vector.tensor_scalar(out=lam16[:, sl], in0=psV[j][:],
                                            scalar1=LLR_CLIP, scalar2=-LLR_CLIP,
                                            op0=op.min, op1=op.max)
                    nc.vector.tensor_copy(V[:, sl], psV[j][:])
                transport(lam16, issc, aginL, agoutL, rshL, rsoL, irsc, lamR)
                # row compute
                nc.scalar.activation(out=u0t[:], in_=lamR[:], func=AF.Tanh, scale=0.5 / 8.0)
                nc.vector.tensor_scalar(out=ut[:], in0=u0t[:], scalar1=UCLIP,
                                        scalar2=-UCLIP, op0=op.min, op1=op.max)
                nc.vector.tensor_reduce(out=Prt[:], in_=ut[:].rearrange("p (m k) -> p m k", k=6),
                                        axis=AX.X, op=op.mult)
                nc.vector.reciprocal(out=rct[:], in_=ut[:])
                nc.vector.tensor_tensor(
                    wct[:].rearrange("p (m k) -> p m k", k=6),
                    Prt[:].unsqueeze(2).broadcast_to([P, ML, 6]),
                    rct[:].rearrange("p (m k) -> p m k", k=6), op.mult)
                nc.vector.tensor_scalar(out=wct[:], in0=wct[:], scalar1=EPS1,
                                        scalar2=-EPS1, op0=op.min, op1=op.max)
                nc.scalar.activation(out=t1t[:], in_=wct[:], func=AF.Ln, bias=1.0, scale=EPS1)
                nc.scalar.activation(out=t2t[:], in_=wct[:], func=AF.Ln, bias=1.0, scale=-EPS1)
                # C-damp: Crow' = (1-g)Crow + gWe*t1 - gWe*t2
                for j in range(3):
                    sl = slice(512 * j, 512 * (j + 1))
                    nc.tensor.matmul(out=psC[j][:], lhsT=dOm16[:], rhs=Crow[:, sl],
                                     start=True, stop=False)
                    nc.tensor.matmul(out=psC[j][:], lhsT=dGWe[:], rhs=t1t[:, sl],
                                     start=False, stop=False)
                    nc.tensor.matmul(out=psC[j][:], lhsT=dGWen[:], rhs=t2t[:, sl],
                                     start=False, stop=True)
                    nc.vector.tensor_copy(Crow[:, sl], psC[j][:])
                transport(Crow, issr, aginR, agoutR, rshR, rsoR, irsr, Ccol)
            # final output
            t = emit_t(T_run)
            emit_out(T_run - 1, t)

    nc.finalize()
    return nc


def kernel(**inputs):
    chn = np.ascontiguousarray(np.asarray(inputs["chn_llr"], np.float32))
    row_idx = np.asarray(inputs["row_idx"])
    col_idx = np.asarray(inputs["col_idx"])
    key = (row_idx.tobytes(), col_idx.tobytes())
    if "tables" not in _cache or _cache.get("key") != key:
        _cache["tables"] = _build_tables(row_idx, col_idx)
        _cache["key"] = key
        _cache.pop("nc", None)
    tb = _cache["tables"]
    PAD = tb["PAD"]
    T_run = int(_cache.get("T_run", T))
    if "nc" not in _cache:
        _cache["nc"] = _build_program(PAD, T_run)
    nc = _cache["nc"]

    w1s = np.stack([np.asarray(inputs[k], np.float32).reshape(20, 1).T * LOG10SCALE
                    for k in ("gW1", "iW1", "eW1")]).astype(np.float32)      # (3,1,20)
    b1s = np.stack([np.asarray(inputs[k], np.float32).reshape(20, 1)
                    for k in ("gb1", "ib1", "eb1")]).astype(np.float32)      # (3,20,1)
    w2s = np.stack([np.asarray(inputs[k], np.float32).reshape(1, 20).T
                    for k in ("gW2", "iW2", "eW2")]).astype(np.float32)      # (3,20,1)
    b2s = np.stack([np.asarray(inputs[k], np.float32).reshape(1, 1)
                    for k in ("gb2", "ib2", "eb2")]).astype(np.float32)      # (3,1,1)
    idn = np.eye(P, dtype=np.float32)

    in_maps = []
    for c in range(NC):
        in_maps.append({
            "chn": chn,
            "chm": np.ascontiguousarray(chn[c * NL:(c + 1) * NL, :]),
            "idn": idn,
            "w1s": w1s, "b1s": b1s, "w2s": w2s, "b2s": b2s,
            "ssc": np.ascontiguousarray(np.tile(tb["ssc"][c][None, :], (P, 1))),
            "rsc": np.ascontiguousarray(np.tile(tb["rsc"][c][None, :], (P, 1))),
            "ssr": np.ascontiguousarray(np.tile(tb["ssr"][c][None, :], (P, 1))),
            "rsr": np.ascontiguousarray(np.tile(tb["rsr"][c][None, :], (P, 1))),
        })
    if "sharded" not in _cache:
        _cache["sharded"] = _make_sharded(nc)
    sharded, in_names, out_names, out_avals, n_params = _cache["sharded"]
    import jax
    import jax.numpy as jnp
    from jax.sharding import NamedSharding, PartitionSpec
    mesh = _cache["mesh"]
    shd = NamedSharding(mesh, PartitionSpec("core"))
    import hashlib
    h = hashlib.sha1()
    for nm in ("chn_llr", "gW1", "gW2", "iW1", "iW2", "eW1", "eW2", "gb1", "ib1", "eb1"):
        h.update(np.ascontiguousarray(np.asarray(inputs[nm])).tobytes())
    ikey = h.hexdigest()
    if _cache.get("dev_key") != ikey:
        per_core = [[np.asarray(in_maps[c][nm]) for nm in in_names[:n_params]]
                    for c in range(NC)]
        concat_in = [np.concatenate([per_core[c][i] for c in range(NC)], axis=0)
                     for i in range(n_params)]
        _cache["dev_in"] = [jax.device_put(a, shd) for a in concat_in]
        _cache["dev_key"] = ikey
    if "dev_zeros" not in _cache:
        _cache["dev_zeros"] = [jnp.zeros((NC * a.shape[0], *a.shape[1:]), a.dtype,
                                         device=shd) for a in out_avals]
    out_arrs = sharded(*_cache["dev_in"], *_cache["dev_zeros"])
    oi = out_names.index("out")
    arr = out_arrs[oi]
    T_r = out_avals[oi].shape[0]
    out = np.empty((T_r, N, B), np.float32)
    from concurrent.futures import ThreadPoolExecutor

    def fetch(s):
        c = s.index[0].start // T_r if s.index[0].start else 0
        out[:, c * NL:(c + 1) * NL, :] = np.asarray(s.data)
        return None

    with ThreadPoolExecutor(NC) as ex:
        list(ex.map(fetch, arr.addressable_shards))
    return out


def _make_sharded(nc):
    import jax
    import jax.numpy as jnp
    from jax.experimental.shard_map import shard_map
    from jax.sharding import Mesh, PartitionSpec
    import concourse.mybir as mybir
    from concourse import bass2jax
    from concourse.bass2jax import _bass_exec_p, partition_id_tensor, install_neuronx_cc_hook

    install_neuronx_cc_hook()
    partition_name = nc.partition_id_tensor.name if nc.partition_id_tensor else None
    in_names, out_names, out_avals = [], [], []
    for alloc in nc.m.functions[0].allocations:
        if not isinstance(alloc, mybir.MemoryLocationSet):
            continue
        name = alloc.memorylocations[0].name
        if alloc.kind == "ExternalInput":
            if name != partition_name:
                in_names.append(name)
        elif alloc.kind == "ExternalOutput":
            shape = tuple(alloc.tensor_shape)
            out_avals.append(jax.core.ShapedArray(shape, mybir.dt.np(alloc.dtype)))
            out_names.append(name)
    n_params = len(in_names)
    n_outs = len(out_avals)
    in_names = in_names + out_names
    if partition_name is not None:
        in_names.append(partition_name)
    donate = ()

    def _body(*args):
        operands = list(args)
        if partition_name is not None:
            operands.append(partition_id_tensor())
        outs = _bass_exec_p.bind(
            *operands,
            out_avals=tuple(out_avals),
            in_names=tuple(in_names),
            out_names=tuple(out_names),
            lowering_input_output_aliases=(),
            sim_require_finite=True,
            sim_require_nnan=True,
            nc=nc,
        )
        return tuple(outs)

    devices = jax.devices()[:NC]
    mesh = Mesh(np.asarray(devices), ("core",))
    _cache["mesh"] = mesh
    sharded = jax.jit(
        shard_map(_body, mesh=mesh,
                  in_specs=(PartitionSpec("core"),) * (n_params + n_outs),
                  out_specs=(PartitionSpec("core"),) * n_outs,
                  check_rep=False),
        donate_argnums=donate, keep_unused=True)
    return sharded, in_names, out_names, out_avals, n_params


# revision 18
# speedup vs baseline: 1.0532x; 1.0532x over previous
"""Trainium2 Bass kernel for nn_AdaBP_Decoder (LDPC adaptive BP, T=30 iters).

Strategy: shard edges/variables/checks across 8 NeuronCores; batch B=128 lives
on the 128 SBUF partitions. Per core: col-domain tensors (128, 1536) hold its
1536 edges sorted by variable (seg3 = strided free reduce); row-domain tensors
hold the 1536 edges of its checks sorted by check (seg6). The two per-iteration
edge reorders (col->row of lam, row->col of C~=We*msg_C2V) go: gpsimd
local_scatter into dest-core-grouped fp16 blocks -> AllGather -> static block
transpose -> ReduceScatter (the two collectives supply all rank awareness, so
no dynamic access patterns are needed; the x8 sum of identical copies is exact
in fp16 and folded into downstream scales) -> local_scatter into domain order.
Check-node update uses the signed
tanh-product/divide form (atanh via two Ln's); damping is done as diagonal
matmuls accumulated in PSUM on the TensorEngine.
"""
import sys
sys.path.insert(0, "/opt/trn_rl_repo")
import numpy as np

N, M, COL_DEG, E, B, T = 4096, 2048, 3, 12288, 128, 30
NC = 8
NL, ML, EL = N // NC, M // NC, E // NC
P = 128

LLR_CLIP = 15.0
UCLIP = float(np.tanh(np.float32(15.0) / 2.0))
EPS1 = 1.0 - 1e-6
LOG10SCALE = 10.0 / float(np.log(10.0))

_cache = {}


def _build_tables(row_idx, col_idx):
    row = np.asarray(row_idx).astype(np.int64)
    col = np.asarray(col_idx).astype(np.int64)
    assert row.shape == (E,) and col.shape == (E,)
    assert np.all(np.bincount(col, minlength=N) == COL_DEG)
    assert np.all(np.bincount(row, minlength=M) == E // M)
    colcore = col // NL
    rowcore = row // ML

    localcol, localrow = [], []
    for c in range(NC):
        eids = np.where(colcore == c)[0]
        localcol.append(eids[np.lexsort((eids, col[eids]))])
    for d in range(NC):
        eids = np.where(rowcore == d)[0]
        localrow.append(eids[np.lexsort((eids, row[eids]))])
    poscol = np.empty(E, np.int64)
    posrow = np.empty(E, np.int64)
    for c in range(NC):
        poscol[localcol[c]] = np.arange(EL)
        posrow[localrow[c]] = np.arange(EL)

    S = [[None] * NC for _ in range(NC)]    # col->row blocks
    Tb = [[None] * NC for _ in range(NC)]   # row->col blocks
    for c in range(NC):
        eids = localcol[c]
        for d in range(NC):
            sel = np.where(rowcore[eids] == d)[0]
            S[c][d] = sel[np.argsort(posrow[eids[sel]])]
    for d in range(NC):
        eids = localrow[d]
        for c in range(NC):
            sel = np.where(colcore[eids] == c)[0]
            Tb[d][c] = sel[np.argsort(poscol[eids[sel]])]

    maxblk = max(len(S[c][d]) for c in range(NC) for d in range(NC))
    PAD = maxblk + (maxblk & 1)
    assert NC * PAD * 32 < (1 << 16), f"PAD={PAD} too large for local_scatter"

    # scatter tables (per core), int16:
    #   ssc[c][i]  : col-pos i -> slot in send buffer (d*PAD + k)       [c2r send]
    #   rsc[d][s]  : send-slot s (8*PAD) -> row-pos (or -1 pad)          [c2r recv]
    #   ssr[d][i]  : row-pos i -> slot (c*PAD + k)                       [r2c send]
    #   rsr[c][s]  : slot s -> col-pos (or -1)                           [r2c recv]
    # per-destination send tables with the sender rank baked in:
    #   ssc8[c][d*EL + i] = c*PAD + k  if col-pos i is the k-th entry of block
    #                       (c->d), else -1   (one scatter per destination d)
    ssc8 = np.full((NC, NC * EL), -1, np.int16)
    ssr8 = np.full((NC, NC * EL), -1, np.int16)
    rsc = np.full((NC, NC * PAD), -1, np.int16)
    rsr = np.full((NC, NC * PAD), -1, np.int16)
    for c in range(NC):
        for d in range(NC):
            blk = S[c][d]
            ssc8[c, d * EL + blk] = (c * PAD + np.arange(len(blk))).astype(np.int16)
    for d in range(NC):
        for c in range(NC):
            blk = Tb[d][c]
            ssr8[d, c * EL + blk] = (d * PAD + np.arange(len(blk))).astype(np.int16)
    for d in range(NC):
        for c in range(NC):
            blk = S[c][d]                      # positions in localcol[c]
            gids = localcol[c][blk]
            rsc[d, c * PAD: c * PAD + len(blk)] = posrow[gids].astype(np.int16)
    for c in range(NC):
        for d in range(NC):
            blk = Tb[d][c]
            gids = localrow[d][blk]
            rsr[c, d * PAD: d * PAD + len(blk)] = poscol[gids].astype(np.int16)

    return dict(PAD=PAD, ssc8=ssc8, rsc=rsc, ssr8=ssr8, rsr=rsr)


def _build_program(PAD, T_run=T):
    import concourse.bass as bass
    import concourse.bacc as bacc
    import concourse.mybir as mybir
    from concourse.tile import TileContext

    dt, op = mybir.dt, mybir.AluOpType
    AF = mybir.ActivationFunctionType
    AX = mybir.AxisListType
    SP = NC * PAD   # send buffer width

    nc = bacc.Bacc(None, target_bir_lowering=False)

    chn_ext = nc.declare_dram_parameter("chn", [N, B], dt.float32, isOutput=False)
    chm_ext = nc.declare_dram_parameter("chm", [NL, B], dt.float32, isOutput=False)
    idn_ext = nc.declare_dram_parameter("idn", [P, P], dt.float32, isOutput=False)
    w1_ext = nc.declare_dram_parameter("w1s", [3, 1, 20], dt.float32, isOutput=False)
    b1_ext = nc.declare_dram_parameter("b1s", [3, 20, 1], dt.float32, isOutput=False)
    w2_ext = nc.declare_dram_parameter("w2s", [3, 20, 1], dt.float32, isOutput=False)
    b2_ext = nc.declare_dram_parameter("b2s", [3, 1, 1], dt.float32, isOutput=False)
    ssc_ext = nc.declare_dram_parameter("ssc", [P, NC * EL], dt.int16, isOutput=False)
    rsc_ext = nc.declare_dram_parameter("rsc", [P, SP], dt.int16, isOutput=False)
    ssr_ext = nc.declare_dram_parameter("ssr", [P, NC * EL], dt.int16, isOutput=False)
    rsr_ext = nc.declare_dram_parameter("rsr", [P, SP], dt.int16, isOutput=False)
    out_ext = nc.declare_dram_parameter("out", [T_run, NL, B], dt.float16, isOutput=True)

    rshL = nc.dram_tensor("rshL", [NC, P, SP], dt.float16)
    rsoL = nc.dram_tensor("rsoL", [P, SP], dt.float16)
    rshR = nc.dram_tensor("rshR", [NC, P, SP], dt.float16)
    rsoR = nc.dram_tensor("rsoR", [P, SP], dt.float16)

    with TileContext(nc) as tc:
        with (
            tc.tile_pool(name="sbuf", bufs=1) as pool,
            tc.tile_pool(name="psum", bufs=1, space="PSUM") as pp,
        ):
            f32, f16, i16 = dt.float32, dt.float16, dt.int16
            # --- persistent tiles
            V = pool.tile([P, EL], f32)
            Ccol = pool.tile([P, EL], f16)
            Crow = pool.tile([P, EL], f16)
            ell = pool.tile([P, NL], f32)
            ident = pool.tile([P, P], f32)
            issc = pool.tile([P, NC * EL], i16)
            irsc = pool.tile([P, SP], i16)
            issr = pool.tile([P, NC * EL], i16)
            irsr = pool.tile([P, SP], i16)
            dOm = pool.tile([P, P], f32)
            dG = pool.tile([P, P], f32)
            dGn = pool.tile([P, P], f32)
            dGWe = pool.tile([P, P], f32)
            dGWen = pool.tile([P, P], f32)
            dOm16 = pool.tile([P, P], f16)
            dGn16 = pool.tile([P, P], f16)
            # working tiles
            lam16 = pool.tile([P, EL], f16)
            lamR = pool.tile([P, EL], f16)
            stg = pool.tile([P, NC * SP], f16, tag="bigshare")
            sbB = pool.tile([P, SP], f16)
            u0t = pool.tile([P, EL], f32)
            ut = pool.tile([P, EL], f32)
            Prt = pool.tile([P, ML], f32)
            rct = pool.tile([P, EL], f32)
            wct = pool.tile([P, EL], f32)
            t1t = pool.tile([P, EL], f32)
            t2t = pool.tile([P, EL], f32)
            s3t = pool.tile([P, NL], f32)
            tts = [pool.tile([P, NL], f32, name=f"tt{j}") for j in range(2)]
            touts = [pool.tile([P, NL], f16, name=f"tout{j}") for j in range(2)]
            # psum tiles: 4x384 (V-damp), 3x512 (C-damp), 1x512 (transposes/misc)
            psV = [pp.tile([P, 384], f32, name=f"psV{j}", tag=f"psV{j}", space="PSUM") for j in range(4)]
            psC = [pp.tile([P, 512], f32, name=f"psC{j}", tag=f"psC{j}", space="PSUM") for j in range(3)]
            psT = pp.tile([P, 512], f32, space="PSUM", tag="psT")

            # --- load inputs
            chn_nat = pool.tile([P, N], f32, tag="bigshare")
            nc.scalar.dma_start(out=chn_nat[:].rearrange("p (k b) -> p k b", b=B), in_=chn_ext[:].rearrange("(k p) b -> p k b", p=P))
            chm_nat = pool.tile([P, 4 * P], f32)
            nc.scalar.dma_start(out=chm_nat[:].rearrange("p (k b) -> p k b", b=B), in_=chm_ext[:].rearrange("(k p) b -> p k b", p=P))
            nc.scalar.dma_start(out=ident[:], in_=idn_ext[:])
            nc.scalar.dma_start(out=issc[:], in_=ssc_ext[:])
            nc.scalar.dma_start(out=irsc[:], in_=rsc_ext[:])
            nc.scalar.dma_start(out=issr[:], in_=ssr_ext[:])
            nc.scalar.dma_start(out=irsr[:], in_=rsr_ext[:])
            w1t = pool.tile([1, 60], f32)  # lhsT layouts (3 nets x 20, 1 partition)
            b1t = pool.tile([20, 3], f32)
            w2t = pool.tile([20, 3], f32)
            b2t = pool.tile([1, 3], f32)
            nc.scalar.dma_start(out=w1t[:].rearrange("a (s j) -> a s j", s=3), in_=w1_ext[:].rearrange("s a j -> a s j"))
            nc.scalar.dma_start(out=b1t[:], in_=b1_ext[:].rearrange("s j a -> j s a"))
            nc.scalar.dma_start(out=w2t[:], in_=w2_ext[:].rearrange("s j a -> j s a"))
            nc.scalar.dma_start(out=b2t[:], in_=b2_ext[:].rearrange("s j a -> j s a"))

            # --- adapter nets: Eng -> snr(ln q) -> 3 tiny MLPs -> gamma/Wi/We
            sq = pool.tile([P, N], f32, tag="bigB")
            nc.scalar.activation(out=sq[:], in_=chn_nat[:], func=AF.Square, scale=1.0 / 64.0)
            part = pool.tile([P, P], f32)
            nc.vector.tensor_reduce(out=part[:], in_=sq[:].rearrange("p (k b) -> p b k", k=N // P),
                                    axis=AX.X, op=op.add)
            ones_col = pool.tile([P, 1], f32)
            nc.vector.memset(ones_col[:], 1.0)
            psE = pp.tile([1, P], f32, space="PSUM", tag="psC0")
            nc.tensor.matmul(out=psE[:], lhsT=ones_col[:], rhs=part[:], start=True, stop=True)
            Eng = pool.tile([1, P], f32)
            nc.vector.tensor_copy(Eng[:], psE[:])
            s1 = pool.tile([1, P], f32)
            nc.scalar.activation(out=s1[:], in_=Eng[:], func=AF.Sqrt, bias=1.0, scale=1.0)
            dn = pool.tile([1, P], f32)
            nc.vector.tensor_scalar(out=dn[:], in0=s1[:], scalar1=1.0, scalar2=2.0,
                                    op0=op.add, op1=op.mult)
            rdn = pool.tile([1, P], f32)
            nc.vector.reciprocal(out=rdn[:], in_=dn[:])
            qq = pool.tile([1, P], f32)
            nc.vector.scalar_tensor_tensor(out=qq[:], in0=Eng[:], scalar=1.0, in1=rdn[:],
                                           op0=op.mult, op1=op.mult)
            lnq = pool.tile([1, P], f32)
            nc.scalar.activation(out=lnq[:], in_=qq[:], func=AF.Ln)

            rows = [pool.tile([1, P], f32, name=f"rows{s}") for s in range(3)]
            psH = pp.tile([20, P], f32, space="PSUM", tag="psC1")
            psO = pp.tile([1, P], f32, space="PSUM", tag="psC2")
            hX = pool.tile([20, P], f32)
            for s in range(3):
                nc.tensor.matmul(out=psH[:], lhsT=w1t[:, 20 * s:20 * (s + 1)], rhs=lnq[:],
                                 start=True, stop=True)
                nc.scalar.activation(out=hX[:], in_=psH[:], func=AF.Relu, bias=b1t[:, s:s + 1])
                nc.tensor.matmul(out=psO[:], lhsT=w2t[:, s:s + 1], rhs=hX[:], start=True, stop=True)
                nc.scalar.activation(out=rows[s][:], in_=psO[:], func=AF.Sigmoid,
                                     bias=b2t[:, s:s + 1])
            # transpose rows -> per-partition columns
            ones11 = pool.tile([1, 1], f32)
            nc.vector.memset(ones11[:], 1.0)
            psPP = pp.tile([P, 3], f32, space="PSUM", tag="psT")
            gpp = pool.tile([P, 1], f32)
            wipp = pool.tile([P, 1], f32)
            wepp = pool.tile([P, 1], f32)
            for s, dst in enumerate([gpp, wipp, wepp]):
                nc.tensor.matmul(out=psPP[:, s:s + 1], lhsT=rows[s][:], rhs=ones11[:],
                                 start=True, stop=True)
                nc.vector.tensor_copy(dst[:], psPP[:, s:s + 1])
            ompp = pool.tile([P, 1], f32)
            nc.vector.tensor_scalar(out=ompp[:], in0=gpp[:], scalar1=-1.0, scalar2=1.0,
                                    op0=op.mult, op1=op.add)
            gwepp = pool.tile([P, 1], f32)
            nc.vector.tensor_tensor(gwepp[:], gpp[:], wepp[:], op.mult)
            gwenpp = pool.tile([P, 1], f32)
            nc.vector.tensor_scalar_mul(out=gwenpp[:], in0=gwepp[:], scalar1=-1.0)
            gnpp = pool.tile([P, 1], f32)
            nc.vector.tensor_scalar_mul(out=gnpp[:], in0=gpp[:], scalar1=-1.0)
            # diag matrices
            nc.vector.tensor_scalar_mul(out=dOm[:], in0=ident[:], scalar1=ompp[:])
            nc.vector.tensor_scalar_mul(out=dG[:], in0=ident[:], scalar1=gpp[:])
            nc.vector.tensor_scalar_mul(out=dGn[:], in0=ident[:], scalar1=gnpp[:])
            nc.vector.tensor_scalar_mul(out=dGWe[:], in0=ident[:], scalar1=gwepp[:])
            nc.vector.tensor_scalar_mul(out=dGWen[:], in0=ident[:], scalar1=gwenpp[:])
            nc.vector.tensor_copy(dOm16[:], dOm[:])
            nc.vector.tensor_copy(dGn16[:], dGn[:])
            # ell = Wi * chn_mine^T
            for k in range(4):
                nc.tensor.transpose(out=psT[:, :P], in_=chm_nat[:, k * P:(k + 1) * P],
                                    identity=ident[:])
                nc.vector.tensor_scalar_mul(out=ell[:, k * P:(k + 1) * P], in0=psT[:, :P],
                                            scalar1=wipp[:])
            # init state
            nc.vector.memset(V[:], 0.0)
            nc.vector.memset(Ccol[:], 0.0)
            nc.vector.memset(Crow[:], 0.0)

            # --- helper emitters
            def emit_t(i):
                t = tts[i % 2]
                nc.vector.tensor_reduce(out=s3t[:], in_=Ccol[:].rearrange("p (v j) -> p v j", j=3),
                                        axis=AX.X, op=op.add)
                nc.vector.tensor_tensor(t[:], s3t[:], ell[:], op.add)
                return t

            def emit_out(i, t):
                tout = touts[i % 2]
                for k in range(4):
                    nc.tensor.transpose(out=psT[:, :P],
                                        in_=t[:, k * P:(k + 1) * P], identity=ident[:])
                    nc.vector.tensor_copy(tout[:, k * P:(k + 1) * P], psT[:, :P])
                nc.scalar.dma_start(
                    out=out_ext[i].rearrange("(k nl) b -> nl k b", k=4),
                    in_=tout[:].rearrange("p (k b) -> p k b", k=4))

            def transport(src16, sidx8, rsh, rso, ridx, dst16):
                # build the zero-padded ReduceScatter input directly: one
                # scatter per destination shard, sender-rank offset baked into
                # the per-core idx tables; untouched slots are zeroed by the
                # scatter, so the cross-core sum is an exact concatenation.
                for d_ in range(NC):
                    nc.gpsimd.local_scatter(
                        out_ap=stg[:, d_ * SP:(d_ + 1) * SP], data_ap=src16[:],
                        idxs_ap=sidx8[:, d_ * EL:(d_ + 1) * EL],
                        channels=P, num_elems=SP, num_idxs=EL)
                nc.scalar.dma_start(out=rsh[:].rearrange("d p f -> p d f"),
                                    in_=stg[:].rearrange("p (d f) -> p d f", d=NC))
                nc.gpsimd.collective_compute(
                    "ReduceScatter", op.add, replica_groups=[list(range(NC))],
                    ins=[rsh[:]], outs=[rso[:]])
                nc.scalar.dma_start(out=sbB[:], in_=rso[:])
                nc.gpsimd.local_scatter(out_ap=dst16[:], data_ap=sbB[:], idxs_ap=ridx[:],
                                        channels=P, num_elems=EL, num_idxs=SP)

            # --- main loop (fully unrolled)
            for i in range(T_run):
                t = emit_t(i)
                if i >= 1:
                    emit_out(i - 1, t)
                # V-damp: V' = (1-g)V + g*t[rep3] - g*Ccol ; lam = clip(V')
                for j in range(4):
                    sl = slice(384 * j, 384 * (j + 1))
                    nc.tensor.matmul(out=psV[j][:], lhsT=dOm[:], rhs=V[:, sl],
                                     start=True, stop=False)
                    nc.tensor.matmul(out=psV[j][:], lhsT=dG[:],
                                     rhs=t[:, 128 * j:128 * (j + 1)].unsqueeze(2)
                                     .broadcast_to([P, 128, 3]),
                                     start=False, stop=False)
                    nc.tensor.matmul(out=psV[j][:], lhsT=dGn16[:], rhs=Ccol[:, sl],
                                     start=False, stop=True)
                    nc.vector.tensor_scalar(out=lam16[:, sl], in0=psV[j][:],
                                            scalar1=LLR_CLIP, scalar2=-LLR_CLIP,
                                            op0=op.min, op1=op.max)
                    nc.vector.tensor_copy(V[:, sl], psV[j][:])
                transport(lam16, issc, rshL, rsoL, irsc, lamR)
                # row compute
                nc.scalar.activation(out=u0t[:], in_=lamR[:], func=AF.Tanh, scale=0.5)
                nc.vector.tensor_scalar(out=ut[:], in0=u0t[:], scalar1=UCLIP,
                                        scalar2=-UCLIP, op0=op.min, op1=op.max)
                nc.vector.tensor_reduce(out=Prt[:], in_=ut[:].rearrange("p (m k) -> p m k", k=6),
                                        axis=AX.X, op=op.mult)
                nc.vector.reciprocal(out=rct[:], in_=ut[:])
                nc.vector.tensor_tensor(
                    wct[:].rearrange("p (m k) -> p m k", k=6),
                    Prt[:].unsqueeze(2).broadcast_to([P, ML, 6]),
                    rct[:].rearrange("p (m k) -> p m k", k=6), op.mult)
                nc.vector.tensor_scalar(out=wct[:], in0=wct[:], scalar1=EPS1,
                                        scalar2=-EPS1, op0=op.min, op1=op.max)
                nc.scalar.activation(out=t1t[:], in_=wct[:], func=AF.Ln, bias=1.0, scale=EPS1)
                nc.scalar.activation(out=t2t[:], in_=wct[:], func=AF.Ln, bias=1.0, scale=-EPS1)
                # C-damp: Crow' = (1-g)Crow + gWe*t1 - gWe*t2
                for j in range(3):
                    sl = slice(512 * j, 512 * (j + 1))
                    nc.tensor.matmul(out=psC[j][:], lhsT=dOm16[:], rhs=Crow[:, sl],
                                     start=True, stop=False)
                    nc.tensor.matmul(out=psC[j][:], lhsT=dGWe[:], rhs=t1t[:, sl],
                                     start=False, stop=False)
                    nc.tensor.matmul(out=psC[j][:], lhsT=dGWen[:], rhs=t2t[:, sl],
                                     start=False, stop=True)
                    nc.vector.tensor_copy(Crow[:, sl], psC[j][:])
                transport(Crow, issr, rshR, rsoR, irsr, Ccol)
            # final output
            t = emit_t(T_run)
            emit_out(T_run - 1, t)

    nc.finalize()
    return nc


def kernel(**inputs):
    chn = np.ascontiguousarray(np.asarray(inputs["chn_llr"], np.float32))
    row_idx = np.asarray(inputs["row_idx"])
    col_idx = np.asarray(inputs["col_idx"])
    key = (row_idx.tobytes(), col_idx.tobytes())
    if "tables" not in _cache or _cache.get("key") != key:
        _cache["tables"] = _build_tables(row_idx, col_idx)
        _cache["key"] = key
        _cache.pop("nc", None)
    tb = _cache["tables"]
    PAD = tb["PAD"]
    T_run = int(_cache.get("T_run", T))
    if "nc" not in _cache:
        _cache["nc"] = _build_program(PAD, T_run)
    nc = _cache["nc"]

    w1s = np.stack([np.asarray(inputs[k], np.float32).reshape(20, 1).T * LOG10SCALE
                    for k in ("gW1", "iW1", "eW1")]).astype(np.float32)      # (3,1,20)
    b1s = np.stack([np.asarray(inputs[k], np.float32).reshape(20, 1)
                    for k in ("gb1", "ib1", "eb1")]).astype(np.float32)      # (3,20,1)
    w2s = np.stack([np.asarray(inputs[k], np.float32).reshape(1, 20).T
                    for k in ("gW2", "iW2", "eW2")]).astype(np.float32)      # (3,20,1)
    b2s = np.stack([np.asarray(inputs[k], np.float32).reshape(1, 1)
                    for k in ("gb2", "ib2", "eb2")]).astype(np.float32)      # (3,1,1)
    idn = np.eye(P, dtype=np.float32)

    in_maps = []
    for c in range(NC):
        in_maps.append({
            "chn": chn,
            "chm": np.ascontiguousarray(chn[c * NL:(c + 1) * NL, :]),
            "idn": idn,
            "w1s": w1s, "b1s": b1s, "w2s": w2s, "b2s": b2s,
            "ssc": np.ascontiguousarray(np.tile(tb["ssc8"][c][None, :], (P, 1))),
            "rsc": np.ascontiguousarray(np.tile(tb["rsc"][c][None, :], (P, 1))),
            "ssr": np.ascontiguousarray(np.tile(tb["ssr8"][c][None, :], (P, 1))),
            "rsr": np.ascontiguousarray(np.tile(tb["rsr"][c][None, :], (P, 1))),
        })
    if "sharded" not in _cache:
        _cache["sharded"] = _make_sharded(nc)
    sharded, in_names, out_names, out_avals, n_params = _cache["sharded"]
    import jax
    import jax.numpy as jnp
    from jax.sharding import NamedSharding, PartitionSpec
    mesh = _cache["mesh"]
    shd = NamedSharding(mesh, PartitionSpec("core"))
    import hashlib
    h = hashlib.sha1()
    for nm in ("chn_llr", "gW1", "gW2", "iW1", "iW2", "eW1", "eW2", "gb1", "ib1", "eb1"):
        h.update(np.ascontiguousarray(np.asarray(inputs[nm])).tobytes())
    ikey = h.hexdigest()
    if _cache.get("dev_key") != ikey:
        per_core = [[np.asarray(in_maps[c][nm]) for nm in in_names[:n_params]]
                    for c in range(NC)]
        concat_in = [np.concatenate([per_core[c][i] for c in range(NC)], axis=0)
                     for i in range(n_params)]
        _cache["dev_in"] = [jax.device_put(a, shd) for a in concat_in]
        _cache["dev_key"] = ikey
    if "dev_zeros" not in _cache:
        _cache["dev_zeros"] = [jnp.zeros((NC * a.shape[0], *a.shape[1:]), a.dtype,
                                         device=shd) for a in out_avals]
    out_arrs = sharded(*_cache["dev_in"], *_cache["dev_zeros"])
    oi = out_names.index("out")
    arr = out_arrs[oi]
    T_r = out_avals[oi].shape[0]
    out = np.empty((T_r, N, B), np.float32)
    from concurrent.futures import ThreadPoolExecutor

    def fetch(s):
        c = s.index[0].start // T_r if s.index[0].start else 0
        out[:, c * NL:(c + 1) * NL, :] = np.asarray(s.data)
        return None

    with ThreadPoolExecutor(NC) as ex:
        list(ex.map(fetch, arr.addressable_shards))
    return out


def _make_sharded(nc):
    import jax
    import jax.numpy as jnp
    from jax.experimental.shard_map import shard_map
    from jax.sharding import Mesh, PartitionSpec
    import concourse.mybir as mybir
    from concourse import bass2jax
    from concourse.bass2jax import _bass_exec_p, partition_id_tensor, install_neuronx_cc_hook

    install_neuronx_cc_hook()
    partition_name = nc.partition_id_tensor.name if nc.partition_id_tensor else None
    in_names, out_names, out_avals = [], [], []
    for alloc in nc.m.functions[0].allocations:
        if not isinstance(alloc, mybir.MemoryLocationSet):
            continue
        name = alloc.memorylocations[0].name
        if alloc.kind == "ExternalInput":
            if name != partition_name:
                in_names.append(name)
        elif alloc.kind == "ExternalOutput":
            shape = tuple(alloc.tensor_shape)
            out_avals.append(jax.core.ShapedArray(shape, mybir.dt.np(alloc.dtype)))
            out_names.append(name)
    n_params = len(in_names)
    n_outs = len(out_avals)
    in_names = in_names + out_names
    if partition_name is not None:
        in_names.append(partition_name)
    donate = ()

    def _body(*args):
        operands = list(args)
        if partition_name is not None:
            operands.append(partition_id_tensor())
        outs = _bass_exec_p.bind(
            *operands,
            out_avals=tuple(out_avals),
            in_names=tuple(in_names),
            out_names=tuple(out_names),
            lowering_input_output_aliases=(),
            sim_require_finite=True,
            sim_require_nnan=True,
            nc=nc,
        )
        return tuple(outs)

    devices = jax.devices()[:NC]
    mesh = Mesh(np.asarray(devices), ("core",))
    _cache["mesh"] = mesh
    sharded = jax.jit(
        shard_map(_body, mesh=mesh,
                  in_specs=(PartitionSpec("core"),) * (n_params + n_outs),
                  out_specs=(PartitionSpec("core"),) * n_outs,
                  check_rep=False),
        donate_argnums=donate, keep_unused=True)
    return sharded, in_names, out_names, out_avals, n_params


# revision 19
# speedup vs baseline: 2.0910x; 1.9854x over previous
"""Trainium2 Bass kernel for nn_AdaBP_Decoder (LDPC adaptive BP, T=30 iters).

Strategy: shard edges/variables/checks across 8 NeuronCores; batch B=128 lives
on the 128 SBUF partitions. Per core: col-domain tensors (128, 1536) hold its
1536 edges sorted by variable (seg3 = strided free reduce); row-domain tensors
hold the 1536 edges of its checks sorted by check (seg6). The two per-iteration
edge reorders (col->row of lam, row->col of C~=We*msg_C2V) go: gpsimd
local_scatter into dest-core-grouped fp16 blocks -> AllGather -> static block
transpose -> ReduceScatter (the two collectives supply all rank awareness, so
no dynamic access patterns are needed; the x8 sum of identical copies is exact
in fp16 and folded into downstream scales) -> local_scatter into domain order.
Check-node update uses the signed
tanh-product/divide form (atanh via two Ln's); damping is done as diagonal
matmuls accumulated in PSUM on the TensorEngine.
"""
import sys
sys.path.insert(0, "/opt/trn_rl_repo")
import numpy as np

N, M, COL_DEG, E, B, T = 4096, 2048, 3, 12288, 128, 30
NC = 8
NL, ML, EL = N // NC, M // NC, E // NC
P = 128

LLR_CLIP = 15.0
UCLIP = float(np.tanh(np.float32(15.0) / 2.0))
EPS1 = 1.0 - 1e-6
LOG10SCALE = 10.0 / float(np.log(10.0))

_cache = {}


def _build_tables(row_idx, col_idx):
    row = np.asarray(row_idx).astype(np.int64)
    col = np.asarray(col_idx).astype(np.int64)
    assert row.shape == (E,) and col.shape == (E,)
    assert np.all(np.bincount(col, minlength=N) == COL_DEG)
    assert np.all(np.bincount(row, minlength=M) == E // M)
    colcore = col // NL
    rowcore = row // ML

    localcol, localrow = [], []
    for c in range(NC):
        eids = np.where(colcore == c)[0]
        localcol.append(eids[np.lexsort((eids, col[eids]))])
    for d in range(NC):
        eids = np.where(rowcore == d)[0]
        localrow.append(eids[np.lexsort((eids, row[eids]))])
    poscol = np.empty(E, np.int64)
    posrow = np.empty(E, np.int64)
    for c in range(NC):
        poscol[localcol[c]] = np.arange(EL)
        posrow[localrow[c]] = np.arange(EL)

    S = [[None] * NC for _ in range(NC)]    # col->row blocks
    Tb = [[None] * NC for _ in range(NC)]   # row->col blocks
    for c in range(NC):
        eids = localcol[c]
        for d in range(NC):
            sel = np.where(rowcore[eids] == d)[0]
            S[c][d] = sel[np.argsort(posrow[eids[sel]])]
    for d in range(NC):
        eids = localrow[d]
        for c in range(NC):
            sel = np.where(colcore[eids] == c)[0]
            Tb[d][c] = sel[np.argsort(poscol[eids[sel]])]

    maxblk = max(len(S[c][d]) for c in range(NC) for d in range(NC))
    PAD = maxblk + (maxblk & 1)
    assert NC * PAD * 32 < (1 << 16), f"PAD={PAD} too large for local_scatter"

    # scatter tables (per core), int16:
    #   ssc[c][i]  : col-pos i -> slot in send buffer (d*PAD + k)       [c2r send]
    #   rsc[d][s]  : send-slot s (8*PAD) -> row-pos (or -1 pad)          [c2r recv]
    #   ssr[d][i]  : row-pos i -> slot (c*PAD + k)                       [r2c send]
    #   rsr[c][s]  : slot s -> col-pos (or -1)                           [r2c recv]
    ssc = np.zeros((NC, EL), np.int16)
    rsc = np.full((NC, NC * PAD), -1, np.int16)
    ssr = np.zeros((NC, EL), np.int16)
    rsr = np.full((NC, NC * PAD), -1, np.int16)
    for c in range(NC):
        for d in range(NC):
            blk = S[c][d]
            ssc[c, blk] = (d * PAD + np.arange(len(blk))).astype(np.int16)
    for d in range(NC):
        for c in range(NC):
            blk = S[c][d]                      # positions in localcol[c]
            gids = localcol[c][blk]
            rsc[d, c * PAD: c * PAD + len(blk)] = posrow[gids].astype(np.int16)
    for d in range(NC):
        for c in range(NC):
            blk = Tb[d][c]
            ssr[d, blk] = (c * PAD + np.arange(len(blk))).astype(np.int16)
    for c in range(NC):
        for d in range(NC):
            blk = Tb[d][c]
            gids = localrow[d][blk]
            rsr[c, d * PAD: d * PAD + len(blk)] = poscol[gids].astype(np.int16)

    return dict(PAD=PAD, ssc=ssc, rsc=rsc, ssr=ssr, rsr=rsr)


def _build_program(PAD, T_run=T):
    import concourse.bass as bass
    import concourse.bacc as bacc
    import concourse.mybir as mybir
    from concourse.tile import TileContext

    dt, op = mybir.dt, mybir.AluOpType
    AF = mybir.ActivationFunctionType
    AX = mybir.AxisListType
    SP = NC * PAD   # send buffer width

    nc = bacc.Bacc(None, target_bir_lowering=False)

    chn_ext = nc.declare_dram_parameter("chn", [N, B], dt.float32, isOutput=False)
    chm_ext = nc.declare_dram_parameter("chm", [NL, B], dt.float32, isOutput=False)
    idn_ext = nc.declare_dram_parameter("idn", [P, P], dt.float32, isOutput=False)
    w1_ext = nc.declare_dram_parameter("w1s", [3, 1, 20], dt.float32, isOutput=False)
    b1_ext = nc.declare_dram_parameter("b1s", [3, 20, 1], dt.float32, isOutput=False)
    w2_ext = nc.declare_dram_parameter("w2s", [3, 20, 1], dt.float32, isOutput=False)
    b2_ext = nc.declare_dram_parameter("b2s", [3, 1, 1], dt.float32, isOutput=False)
    ssc_ext = nc.declare_dram_parameter("ssc", [P, EL], dt.int16, isOutput=False)
    rsc_ext = nc.declare_dram_parameter("rsc", [P, SP], dt.int16, isOutput=False)
    ssr_ext = nc.declare_dram_parameter("ssr", [P, EL], dt.int16, isOutput=False)
    rsr_ext = nc.declare_dram_parameter("rsr", [P, SP], dt.int16, isOutput=False)
    out_ext = nc.declare_dram_parameter("out", [T_run, NL, B], dt.float16, isOutput=True)

    aginL = nc.dram_tensor("aginL", [P, SP], dt.float16)
    agoutL = nc.dram_tensor("agoutL", [NC, P, SP], dt.float16, addr_space="Shared")
    rshL = nc.dram_tensor("rshL", [NC, P, SP], dt.float16)
    rsoL = nc.dram_tensor("rsoL", [P, SP], dt.float16)
    aginR = nc.dram_tensor("aginR", [P, SP], dt.float16)
    agoutR = nc.dram_tensor("agoutR", [NC, P, SP], dt.float16, addr_space="Shared")
    rshR = nc.dram_tensor("rshR", [NC, P, SP], dt.float16)
    rsoR = nc.dram_tensor("rsoR", [P, SP], dt.float16)

    with TileContext(nc) as tc:
        with (
            tc.tile_pool(name="sbuf", bufs=1) as pool,
            tc.tile_pool(name="psum", bufs=1, space="PSUM") as pp,
        ):
            f32, f16, i16 = dt.float32, dt.float16, dt.int16
            # --- persistent tiles
            V = pool.tile([P, EL], f32)
            Ccol = pool.tile([P, EL], f16)
            Crow = pool.tile([P, EL], f16)
            ell = pool.tile([P, NL], f32)
            ident = pool.tile([P, P], f32)
            issc = pool.tile([P, EL], i16)
            irsc = pool.tile([P, SP], i16)
            issr = pool.tile([P, EL], i16)
            irsr = pool.tile([P, SP], i16)
            dOm = pool.tile([P, P], f32)
            dG = pool.tile([P, P], f32)
            dGn = pool.tile([P, P], f32)
            dGWe = pool.tile([P, P], f32)
            dGWen = pool.tile([P, P], f32)
            dOm16 = pool.tile([P, P], f16)
            dGn16 = pool.tile([P, P], f16)
            # working tiles
            lam16 = pool.tile([P, EL], f16)
            lamR = pool.tile([P, EL], f16)
            sbA = pool.tile([P, SP], f16)
            sbB = pool.tile([P, SP], f16)
            u0t = pool.tile([P, EL], f32)
            ut = pool.tile([P, EL], f32)
            Prt = pool.tile([P, ML], f32)
            rct = pool.tile([P, EL], f32)
            wct = pool.tile([P, EL], f32)
            t1t = pool.tile([P, EL], f32)
            t2t = pool.tile([P, EL], f32)
            s3t = pool.tile([P, NL], f32)
            tts = [pool.tile([P, NL], f32, name=f"tt{j}") for j in range(2)]
            touts = [pool.tile([P, NL], f16, name=f"tout{j}") for j in range(2)]
            # psum tiles: 4x384 (V-damp), 3x512 (C-damp), 1x512 (transposes/misc)
            psV = [pp.tile([P, 384], f32, name=f"psV{j}", tag=f"psV{j}", space="PSUM") for j in range(4)]
            psC = [pp.tile([P, 512], f32, name=f"psC{j}", tag=f"psC{j}", space="PSUM") for j in range(3)]
            psT = pp.tile([P, 512], f32, space="PSUM", tag="psT")

            # --- load inputs
            chn_nat = pool.tile([P, N], f32, tag="bigA")
            nc.scalar.dma_start(out=chn_nat[:].rearrange("p (k b) -> p k b", b=B), in_=chn_ext[:].rearrange("(k p) b -> p k b", p=P))
            chm_nat = pool.tile([P, 4 * P], f32)
            nc.scalar.dma_start(out=chm_nat[:].rearrange("p (k b) -> p k b", b=B), in_=chm_ext[:].rearrange("(k p) b -> p k b", p=P))
            nc.scalar.dma_start(out=ident[:], in_=idn_ext[:])
            nc.scalar.dma_start(out=issc[:], in_=ssc_ext[:])
            nc.scalar.dma_start(out=irsc[:], in_=rsc_ext[:])
            nc.scalar.dma_start(out=issr[:], in_=ssr_ext[:])
            nc.scalar.dma_start(out=irsr[:], in_=rsr_ext[:])
            w1t = pool.tile([1, 60], f32)  # lhsT layouts (3 nets x 20, 1 partition)
            b1t = pool.tile([20, 3], f32)
            w2t = pool.tile([20, 3], f32)
            b2t = pool.tile([1, 3], f32)
            nc.scalar.dma_start(out=w1t[:].rearrange("a (s j) -> a s j", s=3), in_=w1_ext[:].rearrange("s a j -> a s j"))
            nc.scalar.dma_start(out=b1t[:], in_=b1_ext[:].rearrange("s j a -> j s a"))
            nc.scalar.dma_start(out=w2t[:], in_=w2_ext[:].rearrange("s j a -> j s a"))
            nc.scalar.dma_start(out=b2t[:], in_=b2_ext[:].rearrange("s j a -> j s a"))

            # --- adapter nets: Eng -> snr(ln q) -> 3 tiny MLPs -> gamma/Wi/We
            sq = pool.tile([P, N], f32, tag="bigB")
            nc.scalar.activation(out=sq[:], in_=chn_nat[:], func=AF.Square, scale=1.0 / 64.0)
            part = pool.tile([P, P], f32)
            nc.vector.tensor_reduce(out=part[:], in_=sq[:].rearrange("p (k b) -> p b k", k=N // P),
                                    axis=AX.X, op=op.add)
            ones_col = pool.tile([P, 1], f32)
            nc.vector.memset(ones_col[:], 1.0)
            psE = pp.tile([1, P], f32, space="PSUM", tag="psC0")
            nc.tensor.matmul(out=psE[:], lhsT=ones_col[:], rhs=part[:], start=True, stop=True)
            Eng = pool.tile([1, P], f32)
            nc.vector.tensor_copy(Eng[:], psE[:])
            s1 = pool.tile([1, P], f32)
            nc.scalar.activation(out=s1[:], in_=Eng[:], func=AF.Sqrt, bias=1.0, scale=1.0)
            dn = pool.tile([1, P], f32)
            nc.vector.tensor_scalar(out=dn[:], in0=s1[:], scalar1=1.0, scalar2=2.0,
                                    op0=op.add, op1=op.mult)
            rdn = pool.tile([1, P], f32)
            nc.vector.reciprocal(out=rdn[:], in_=dn[:])
            qq = pool.tile([1, P], f32)
            nc.vector.scalar_tensor_tensor(out=qq[:], in0=Eng[:], scalar=1.0, in1=rdn[:],
                                           op0=op.mult, op1=op.mult)
            lnq = pool.tile([1, P], f32)
            nc.scalar.activation(out=lnq[:], in_=qq[:], func=AF.Ln)

            rows = [pool.tile([1, P], f32, name=f"rows{s}") for s in range(3)]
            psH = pp.tile([20, P], f32, space="PSUM", tag="psC1")
            psO = pp.tile([1, P], f32, space="PSUM", tag="psC2")
            hX = pool.tile([20, P], f32)
            for s in range(3):
                nc.tensor.matmul(out=psH[:], lhsT=w1t[:, 20 * s:20 * (s + 1)], rhs=lnq[:],
                                 start=True, stop=True)
                nc.scalar.activation(out=hX[:], in_=psH[:], func=AF.Relu, bias=b1t[:, s:s + 1])
                nc.tensor.matmul(out=psO[:], lhsT=w2t[:, s:s + 1], rhs=hX[:], start=True, stop=True)
                nc.scalar.activation(out=rows[s][:], in_=psO[:], func=AF.Sigmoid,
                                     bias=b2t[:, s:s + 1])
            # transpose rows -> per-partition columns
            ones11 = pool.tile([1, 1], f32)
            nc.vector.memset(ones11[:], 1.0)
            psPP = pp.tile([P, 3], f32, space="PSUM", tag="psT")
            gpp = pool.tile([P, 1], f32)
            wipp = pool.tile([P, 1], f32)
            wepp = pool.tile([P, 1], f32)
            for s, dst in enumerate([gpp, wipp, wepp]):
                nc.tensor.matmul(out=psPP[:, s:s + 1], lhsT=rows[s][:], rhs=ones11[:],
                                 start=True, stop=True)
                nc.vector.tensor_copy(dst[:], psPP[:, s:s + 1])
            ompp = pool.tile([P, 1], f32)
            nc.vector.tensor_scalar(out=ompp[:], in0=gpp[:], scalar1=-1.0, scalar2=1.0,
                                    op0=op.mult, op1=op.add)
            gwepp = pool.tile([P, 1], f32)
            nc.vector.tensor_tensor(gwepp[:], gpp[:], wepp[:], op.mult)
            gwenpp = pool.tile([P, 1], f32)
            nc.vector.tensor_scalar_mul(out=gwenpp[:], in0=gwepp[:], scalar1=-1.0)
            gnpp = pool.tile([P, 1], f32)
            nc.vector.tensor_scalar_mul(out=gnpp[:], in0=gpp[:], scalar1=-0.125)
            # diag matrices
            nc.vector.tensor_scalar_mul(out=dOm[:], in0=ident[:], scalar1=ompp[:])
            nc.vector.tensor_scalar_mul(out=dG[:], in0=ident[:], scalar1=gpp[:])
            nc.vector.tensor_scalar_mul(out=dGn[:], in0=ident[:], scalar1=gnpp[:])
            nc.vector.tensor_scalar_mul(out=dGWe[:], in0=ident[:], scalar1=gwepp[:])
            nc.vector.tensor_scalar_mul(out=dGWen[:], in0=ident[:], scalar1=gwenpp[:])
            nc.vector.tensor_copy(dOm16[:], dOm[:])
            nc.vector.tensor_copy(dGn16[:], dGn[:])
            # ell = Wi * chn_mine^T
            for k in range(4):
                nc.tensor.transpose(out=psT[:, :P], in_=chm_nat[:, k * P:(k + 1) * P],
                                    identity=ident[:])
                nc.vector.tensor_scalar_mul(out=ell[:, k * P:(k + 1) * P], in0=psT[:, :P],
                                            scalar1=wipp[:])
            # init state
            nc.vector.memset(V[:], 0.0)
            nc.vector.memset(Ccol[:], 0.0)
            nc.vector.memset(Crow[:], 0.0)

            # --- helper emitters
            def emit_t(i):
                t = tts[i % 2]
                nc.vector.tensor_reduce(out=s3t[:], in_=Ccol[:].rearrange("p (v j) -> p v j", j=3),
                                        axis=AX.X, op=op.add)
                nc.vector.scalar_tensor_tensor(out=t[:], in0=s3t[:], scalar=0.125, in1=ell[:], op0=op.mult, op1=op.add)
                return t

            def emit_out(i, t):
                tout = touts[i % 2]
                for k in range(4):
                    nc.tensor.transpose(out=psT[:, :P],
                                        in_=t[:, k * P:(k + 1) * P], identity=ident[:])
                    nc.vector.tensor_copy(tout[:, k * P:(k + 1) * P], psT[:, :P])
                nc.scalar.dma_start(
                    out=out_ext[i].rearrange("(k nl) b -> nl k b", k=4),
                    in_=tout[:].rearrange("p (k b) -> p k b", k=4))

            def transport(src16, sidx, agin, agout, rsh, rso, ridx, dst16):
                nc.gpsimd.local_scatter(out_ap=sbA[:], data_ap=src16[:], idxs_ap=sidx[:],
                                        channels=P, num_elems=SP, num_idxs=EL)
                nc.scalar.dma_start(out=agin[:], in_=sbA[:])
                nc.gpsimd.collective_compute(
                    "AllGather", op.bypass, replica_groups=[list(range(NC))],
                    ins=[agin[:]], outs=[agout[:]])
                # static block transpose: rsh[d, p, c*PAD:...] = agout[c, p, d*PAD:...]
                for c_ in range(NC):
                    nc.scalar.dma_start(
                        out=rsh[:, :, c_ * PAD:(c_ + 1) * PAD],
                        in_=agout[c_].rearrange("p (d f) -> d p f", d=NC))
                nc.gpsimd.collective_compute(
                    "ReduceScatter", op.add, replica_groups=[list(range(NC))],
                    ins=[rsh[:]], outs=[rso[:]])
                nc.scalar.dma_start(out=sbB[:], in_=rso[:])
                nc.gpsimd.local_scatter(out_ap=dst16[:], data_ap=sbB[:], idxs_ap=ridx[:],
                                        channels=P, num_elems=EL, num_idxs=SP)

            # --- main loop (fully unrolled)
            for i in range(T_run):
                t = emit_t(i)
                if i >= 1:
                    emit_out(i - 1, t)
                # V-damp: V' = (1-g)V + g*t[rep3] - g*Ccol ; lam = clip(V')
                for j in range(4):
                    sl = slice(384 * j, 384 * (j + 1))
                    nc.tensor.matmul(out=psV[j][:], lhsT=dOm[:], rhs=V[:, sl],
                                     start=True, stop=False)
                    nc.tensor.matmul(out=psV[j][:], lhsT=dG[:],
                                     rhs=t[:, 128 * j:128 * (j + 1)].unsqueeze(2)
                                     .broadcast_to([P, 128, 3]),
                                     start=False, stop=False)
                    nc.tensor.matmul(out=psV[j][:], lhsT=dGn16[:], rhs=Ccol[:, sl],
                                     start=False, stop=True)
                    nc.vector.tensor_scalar(out=lam16[:, sl], in0=psV[j][:],
                                            scalar1=LLR_CLIP, scalar2=-LLR_CLIP,
                                            op0=op.min, op1=op.max)
                    nc.vector.tensor_copy(V[:, sl], psV[j][:])
                transport(lam16, issc, aginL, agoutL, rshL, rsoL, irsc, lamR)
                # row compute
                nc.scalar.activation(out=u0t[:], in_=lamR[:], func=AF.Tanh, scale=0.5 / 8.0)
                nc.vector.tensor_scalar(out=ut[:], in0=u0t[:], scalar1=UCLIP,
                                        scalar2=-UCLIP, op0=op.min, op1=op.max)
                nc.vector.tensor_reduce(out=Prt[:], in_=ut[:].rearrange("p (m k) -> p m k", k=6),
                                        axis=AX.X, op=op.mult)
                nc.vector.reciprocal(out=rct[:], in_=ut[:])
                nc.vector.tensor_tensor(
                    wct[:].rearrange("p (m k) -> p m k", k=6),
                    Prt[:].unsqueeze(2).broadcast_to([P, ML, 6]),
                    rct[:].rearrange("p (m k) -> p m k", k=6), op.mult)
                nc.vector.tensor_scalar(out=wct[:], in0=wct[:], scalar1=EPS1,
                                        scalar2=-EPS1, op0=op.min, op1=op.max)
                nc.scalar.activation(out=t1t[:], in_=wct[:], func=AF.Ln, bias=1.0, scale=EPS1)
                nc.scalar.activation(out=t2t[:], in_=wct[:], func=AF.Ln, bias=1.0, scale=-EPS1)
                # C-damp: Crow' = (1-g)Crow + gWe*t1 - gWe*t2
                for j in range(3):
                    sl = slice(512 * j, 512 * (j + 1))
                    nc.tensor.matmul(out=psC[j][:], lhsT=dOm16[:], rhs=Crow[:, sl],
                                     start=True, stop=False)
                    nc.tensor.matmul(out=psC[j][:], lhsT=dGWe[:], rhs=t1t[:, sl],
                                     start=False, stop=False)
                    nc.tensor.matmul(out=psC[j][:], lhsT=dGWen[:], rhs=t2t[:, sl],
                                     start=False, stop=True)
                    nc.vector.tensor_copy(Crow[:, sl], psC[j][:])
                transport(Crow, issr, aginR, agoutR, rshR, rsoR, irsr, Ccol)
            # final output
            t = emit_t(T_run)
            emit_out(T_run - 1, t)

    nc.finalize()
    return nc


def kernel(**inputs):
    chn = np.ascontiguousarray(np.asarray(inputs["chn_llr"], np.float32))
    row_idx = np.asarray(inputs["row_idx"])
    col_idx = np.asarray(inputs["col_idx"])
    key = (row_idx.tobytes(), col_idx.tobytes())
    if "tables" not in _cache or _cache.get("key") != key:
        _cache["tables"] = _build_tables(row_idx, col_idx)
        _cache["key"] = key
        _cache.pop("nc", None)
    tb = _cache["tables"]
    PAD = tb["PAD"]
    T_run = int(_cache.get("T_run", T))
    if "nc" not in _cache:
        _cache["nc"] = _build_program(PAD, T_run)
    nc = _cache["nc"]

    w1s = np.stack([np.asarray(inputs[k], np.float32).reshape(20, 1).T * LOG10SCALE
                    for k in ("gW1", "iW1", "eW1")]).astype(np.float32)      # (3,1,20)
    b1s = np.stack([np.asarray(inputs[k], np.float32).reshape(20, 1)
                    for k in ("gb1", "ib1", "eb1")]).astype(np.float32)      # (3,20,1)
    w2s = np.stack([np.asarray(inputs[k], np.float32).reshape(1, 20).T
                    for k in ("gW2", "iW2", "eW2")]).astype(np.float32)      # (3,20,1)
    b2s = np.stack([np.asarray(inputs[k], np.float32).reshape(1, 1)
                    for k in ("gb2", "ib2", "eb2")]).astype(np.float32)      # (3,1,1)
    idn = np.eye(P, dtype=np.float32)

    in_maps = []
    for c in range(NC):
        in_maps.append({
            "chn": chn,
            "chm": np.ascontiguousarray(chn[c * NL:(c + 1) * NL, :]),
            "idn": idn,
            "w1s": w1s, "b1s": b1s, "w2s": w2s, "b2s": b2s,
            "ssc": np.ascontiguousarray(np.tile(tb["ssc"][c][None, :], (P, 1))),
            "rsc": np.ascontiguousarray(np.tile(tb["rsc"][c][None, :], (P, 1))),
            "ssr": np.ascontiguousarray(np.tile(tb["ssr"][c][None, :], (P, 1))),
            "rsr": np.ascontiguousarray(np.tile(tb["rsr"][c][None, :], (P, 1))),
        })
    if "sharded" not in _cache:
        _cache["sharded"] = _make_sharded(nc)
    sharded, in_names, out_names, out_avals, n_params = _cache["sharded"]
    import jax
    import jax.numpy as jnp
    from jax.sharding import NamedSharding, PartitionSpec
    mesh = _cache["mesh"]
    shd = NamedSharding(mesh, PartitionSpec("core"))
    import hashlib
    h = hashlib.sha1()
    for nm in ("chn_llr", "gW1", "gW2", "iW1", "iW2", "eW1", "eW2", "gb1", "ib1", "eb1"):
        h.update(np.ascontiguousarray(np.asarray(inputs[nm])).tobytes())
    ikey = h.hexdigest()
    if _cache.get("dev_key") != ikey:
        per_core = [[np.asarray(in_maps[c][nm]) for nm in in_names[:n_params]]
                    for c in range(NC)]
        concat_in = [np.concatenate([per_core[c][i] for c in range(NC)], axis=0)
                     for i in range(n_params)]
        _cache["dev_in"] = [jax.device_put(a, shd) for a in concat_in]
        _cache["dev_key"] = ikey
    if "dev_zeros" not in _cache:
        _cache["dev_zeros"] = [jnp.zeros((NC * a.shape[0], *a.shape[1:]), a.dtype,
                                         device=shd) for a in out_avals]
    out_arrs = sharded(*_cache["dev_in"], *_cache["dev_zeros"])
    oi = out_names.index("out")
    arr = out_arrs[oi]
    T_r = out_avals[oi].shape[0]
    out = np.empty((T_r, N, B), np.float32)
    from concurrent.futures import ThreadPoolExecutor

    def fetch(s):
        c = s.index[0].start // T_r if s.index[0].start else 0
        out[:, c * NL:(c + 1) * NL, :] = np.asarray(s.data)
        return None

    with ThreadPoolExecutor(NC) as ex:
        list(ex.map(fetch, arr.addressable_shards))
    return out


def _make_sharded(nc):
    import jax
    import jax.numpy as jnp
    from jax.experimental.shard_map import shard_map
    from jax.sharding import Mesh, PartitionSpec
    import concourse.mybir as mybir
    from concourse import bass2jax
    from concourse.bass2jax import _bass_exec_p, partition_id_tensor, install_neuronx_cc_hook

    install_neuronx_cc_hook()
    partition_name = nc.partition_id_tensor.name if nc.partition_id_tensor else None
    in_names, out_names, out_avals = [], [], []
    for alloc in nc.m.functions[0].allocations:
        if not isinstance(alloc, mybir.MemoryLocationSet):
            continue
        name = alloc.memorylocations[0].name
        if alloc.kind == "ExternalInput":
            if name != partition_name:
                in_names.append(name)
        elif alloc.kind == "ExternalOutput":
            shape = tuple(alloc.tensor_shape)
            out_avals.append(jax.core.ShapedArray(shape, mybir.dt.np(alloc.dtype)))
            out_names.append(name)
    n_params = len(in_names)
    n_outs = len(out_avals)
    in_names = in_names + out_names
    if partition_name is not None:
        in_names.append(partition_name)
    donate = ()

    def _body(*args):
        operands = list(args)
        if partition_name is not None:
            operands.append(partition_id_tensor())
        outs = _bass_exec_p.bind(
            *operands,
            out_avals=tuple(out_avals),
            in_names=tuple(in_names),
            out_names=tuple(out_names),
            lowering_input_output_aliases=(),
            sim_require_finite=True,
            sim_require_nnan=True,
            nc=nc,
        )
        return tuple(outs)

    devices = jax.devices()[:NC]
    mesh = Mesh(np.asarray(devices), ("core",))
    _cache["mesh"] = mesh
    sharded = jax.jit(
        shard_map(_body, mesh=mesh,
                  in_specs=(PartitionSpec("core"),) * (n_params + n_outs),
                  out_specs=(PartitionSpec("core"),) * n_outs,
                  check_rep=False),
        donate_argnums=donate, keep_unused=True)
    return sharded, in_names, out_names, out_avals, n_params


# revision 20
# speedup vs baseline: 3.2977x; 1.5771x over previous
"""Trainium2 Bass kernel for nn_AdaBP_Decoder (LDPC adaptive BP, T=30 iters).

Strategy: shard edges/variables/checks across 8 NeuronCores; batch B=128 lives
on the 128 SBUF partitions. Per core: col-domain tensors (128, 1536) hold its
1536 edges sorted by variable (seg3 = strided free reduce); row-domain tensors
hold the 1536 edges of its checks sorted by check (seg6). The two per-iteration
edge reorders (col->row of lam, row->col of C~=We*msg_C2V) go: gpsimd
local_scatter into dest-core-grouped fp16 blocks -> AllGather -> static block
transpose -> ReduceScatter (the two collectives supply all rank awareness, so
no dynamic access patterns are needed; the x8 sum of identical copies is exact
in fp16 and folded into downstream scales) -> local_scatter into domain order.
Check-node update uses the signed
tanh-product/divide form (atanh via two Ln's); damping is done as diagonal
matmuls accumulated in PSUM on the TensorEngine.
"""
import sys
sys.path.insert(0, "/opt/trn_rl_repo")
import numpy as np

N, M, COL_DEG, E, B, T = 4096, 2048, 3, 12288, 128, 30
NC = 8
NL, ML, EL = N // NC, M // NC, E // NC
P = 128

LLR_CLIP = 15.0
UCLIP = float(np.tanh(np.float32(15.0) / 2.0))
EPS1 = 1.0 - 1e-6
LOG10SCALE = 10.0 / float(np.log(10.0))

_cache = {}


def _build_tables(row_idx, col_idx):
    row = np.asarray(row_idx).astype(np.int64)
    col = np.asarray(col_idx).astype(np.int64)
    assert row.shape == (E,) and col.shape == (E,)
    assert np.all(np.bincount(col, minlength=N) == COL_DEG)
    assert np.all(np.bincount(row, minlength=M) == E // M)
    colcore = col // NL
    rowcore = row // ML

    localcol, localrow = [], []
    for c in range(NC):
        eids = np.where(colcore == c)[0]
        localcol.append(eids[np.lexsort((eids, col[eids]))])
    for d in range(NC):
        eids = np.where(rowcore == d)[0]
        localrow.append(eids[np.lexsort((eids, row[eids]))])
    poscol = np.empty(E, np.int64)
    posrow = np.empty(E, np.int64)
    for c in range(NC):
        poscol[localcol[c]] = np.arange(EL)
        posrow[localrow[c]] = np.arange(EL)

    S = [[None] * NC for _ in range(NC)]    # col->row blocks
    Tb = [[None] * NC for _ in range(NC)]   # row->col blocks
    for c in range(NC):
        eids = localcol[c]
        for d in range(NC):
            sel = np.where(rowcore[eids] == d)[0]
            S[c][d] = sel[np.argsort(posrow[eids[sel]])]
    for d in range(NC):
        eids = localrow[d]
        for c in range(NC):
            sel = np.where(colcore[eids] == c)[0]
            Tb[d][c] = sel[np.argsort(poscol[eids[sel]])]

    maxblk = max(len(S[c][d]) for c in range(NC) for d in range(NC))
    PAD = maxblk + (maxblk & 1)
    assert NC * PAD * 32 < (1 << 16), f"PAD={PAD} too large for local_scatter"

    # scatter tables (per core), int16:
    #   ssc[c][i]  : col-pos i -> slot in send buffer (d*PAD + k)       [c2r send]
    #   rsc[d][s]  : send-slot s (8*PAD) -> row-pos (or -1 pad)          [c2r recv]
    #   ssr[d][i]  : row-pos i -> slot (c*PAD + k)                       [r2c send]
    #   rsr[c][s]  : slot s -> col-pos (or -1)                           [r2c recv]
    ssc = np.zeros((NC, EL), np.int16)
    rsc = np.full((NC, NC * PAD), -1, np.int16)
    ssr = np.zeros((NC, EL), np.int16)
    rsr = np.full((NC, NC * PAD), -1, np.int16)
    for c in range(NC):
        for d in range(NC):
            blk = S[c][d]
            ssc[c, blk] = (d * PAD + np.arange(len(blk))).astype(np.int16)
    for d in range(NC):
        for c in range(NC):
            blk = S[c][d]                      # positions in localcol[c]
            gids = localcol[c][blk]
            rsc[d, c * PAD: c * PAD + len(blk)] = posrow[gids].astype(np.int16)
    for d in range(NC):
        for c in range(NC):
            blk = Tb[d][c]
            ssr[d, blk] = (c * PAD + np.arange(len(blk))).astype(np.int16)
    for c in range(NC):
        for d in range(NC):
            blk = Tb[d][c]
            gids = localrow[d][blk]
            rsr[c, d * PAD: d * PAD + len(blk)] = poscol[gids].astype(np.int16)

    return dict(PAD=PAD, ssc=ssc, rsc=rsc, ssr=ssr, rsr=rsr)


def _build_program(PAD, T_run=T):
    import concourse.bass as bass
    import concourse.bacc as bacc
    import concourse.mybir as mybir
    from concourse.tile import TileContext

    dt, op = mybir.dt, mybir.AluOpType
    AF = mybir.ActivationFunctionType
    AX = mybir.AxisListType
    SP = NC * PAD   # send buffer width

    nc = bacc.Bacc(None, target_bir_lowering=False)

    chn_ext = nc.declare_dram_parameter("chn", [N, B], dt.float32, isOutput=False)
    chm_ext = nc.declare_dram_parameter("chm", [NL, B], dt.float32, isOutput=False)
    idn_ext = nc.declare_dram_parameter("idn", [P, P], dt.float32, isOutput=False)
    w1_ext = nc.declare_dram_parameter("w1s", [3, 1, 20], dt.float32, isOutput=False)
    b1_ext = nc.declare_dram_parameter("b1s", [3, 20, 1], dt.float32, isOutput=False)
    w2_ext = nc.declare_dram_parameter("w2s", [3, 20, 1], dt.float32, isOutput=False)
    b2_ext = nc.declare_dram_parameter("b2s", [3, 1, 1], dt.float32, isOutput=False)
    ssc_ext = nc.declare_dram_parameter("ssc", [P, EL], dt.int16, isOutput=False)
    rsc_ext = nc.declare_dram_parameter("rsc", [P, SP], dt.int16, isOutput=False)
    ssr_ext = nc.declare_dram_parameter("ssr", [P, EL], dt.int16, isOutput=False)
    rsr_ext = nc.declare_dram_parameter("rsr", [P, SP], dt.int16, isOutput=False)
    out_ext = nc.declare_dram_parameter("out", [T_run, NL, B], dt.float16, isOutput=True)

    aginL = nc.dram_tensor("aginL", [P, SP], dt.float16)
    agoutL = nc.dram_tensor("agoutL", [NC, P, SP], dt.float16, addr_space="Shared")
    rshL = nc.dram_tensor("rshL", [NC, P, SP], dt.float16)
    rsoL = nc.dram_tensor("rsoL", [P, SP], dt.float16)
    aginR = nc.dram_tensor("aginR", [P, SP], dt.float16)
    agoutR = nc.dram_tensor("agoutR", [NC, P, SP], dt.float16, addr_space="Shared")
    rshR = nc.dram_tensor("rshR", [NC, P, SP], dt.float16)
    rsoR = nc.dram_tensor("rsoR", [P, SP], dt.float16)

    with TileContext(nc) as tc:
        with (
            tc.tile_pool(name="sbuf", bufs=1) as pool,
            tc.tile_pool(name="psum", bufs=1, space="PSUM") as pp,
        ):
            f32, f16, i16 = dt.float32, dt.float16, dt.int16
            # --- persistent tiles
            V = pool.tile([P, EL], f32)
            Ccol = pool.tile([P, EL], f16)
            Crow = pool.tile([P, EL], f16)
            ell = pool.tile([P, NL], f32)
            ident = pool.tile([P, P], f32)
            issc = pool.tile([P, EL], i16)
            irsc = pool.tile([P, SP], i16)
            issr = pool.tile([P, EL], i16)
            irsr = pool.tile([P, SP], i16)
            dOm = pool.tile([P, P], f32)
            dG = pool.tile([P, P], f32)
            dGn = pool.tile([P, P], f32)
            dGWe = pool.tile([P, P], f32)
            dGWen = pool.tile([P, P], f32)
            dOm16 = pool.tile([P, P], f16)
            dGn16 = pool.tile([P, P], f16)
            # working tiles
            lam16 = pool.tile([P, EL], f16)
            lamR = pool.tile([P, EL], f16)
            sbA = pool.tile([P, SP], f16)
            sbB = pool.tile([P, SP], f16)
            u0t = pool.tile([P, EL], f32)
            ut = pool.tile([P, EL], f32)
            Prt = pool.tile([P, ML], f32)
            rct = pool.tile([P, EL], f32)
            wct = pool.tile([P, EL], f32)
            t1t = pool.tile([P, EL], f32)
            t2t = pool.tile([P, EL], f32)
            s3t = pool.tile([P, NL], f32)
            t3t = pool.tile([P, EL], f32)
            tts = [pool.tile([P, NL], f32, name=f"tt{j}") for j in range(2)]
            touts = [pool.tile([P, NL], f16, name=f"tout{j}") for j in range(2)]
            # psum tiles: 4x384 (V-damp), 3x512 (C-damp), 1x512 (transposes/misc)
            psV = [pp.tile([P, 512], f32, name=f"psV{j}", tag=f"psV{j}", space="PSUM") for j in range(3)]
            psC = [pp.tile([P, 512], f32, name=f"psC{j}", tag=f"psC{j}", space="PSUM") for j in range(3)]
            psT = pp.tile([P, 512], f32, space="PSUM", tag="psT")

            # --- load inputs
            chn_nat = pool.tile([P, N], f32, tag="bigA")
            nc.scalar.dma_start(out=chn_nat[:].rearrange("p (k b) -> p k b", b=B), in_=chn_ext[:].rearrange("(k p) b -> p k b", p=P))
            chm_nat = pool.tile([P, 4 * P], f32)
            nc.scalar.dma_start(out=chm_nat[:].rearrange("p (k b) -> p k b", b=B), in_=chm_ext[:].rearrange("(k p) b -> p k b", p=P))
            nc.scalar.dma_start(out=ident[:], in_=idn_ext[:])
            nc.scalar.dma_start(out=issc[:], in_=ssc_ext[:])
            nc.scalar.dma_start(out=irsc[:], in_=rsc_ext[:])
            nc.scalar.dma_start(out=issr[:], in_=ssr_ext[:])
            nc.scalar.dma_start(out=irsr[:], in_=rsr_ext[:])
            w1t = pool.tile([1, 60], f32)  # lhsT layouts (3 nets x 20, 1 partition)
            b1t = pool.tile([20, 3], f32)
            w2t = pool.tile([20, 3], f32)
            b2t = pool.tile([1, 3], f32)
            nc.scalar.dma_start(out=w1t[:].rearrange("a (s j) -> a s j", s=3), in_=w1_ext[:].rearrange("s a j -> a s j"))
            nc.scalar.dma_start(out=b1t[:], in_=b1_ext[:].rearrange("s j a -> j s a"))
            nc.scalar.dma_start(out=w2t[:], in_=w2_ext[:].rearrange("s j a -> j s a"))
            nc.scalar.dma_start(out=b2t[:], in_=b2_ext[:].rearrange("s j a -> j s a"))

            # --- adapter nets: Eng -> snr(ln q) -> 3 tiny MLPs -> gamma/Wi/We
            sq = pool.tile([P, N], f32, tag="bigB")
            nc.scalar.activation(out=sq[:], in_=chn_nat[:], func=AF.Square, scale=1.0 / 64.0)
            part = pool.tile([P, P], f32)
            nc.vector.tensor_reduce(out=part[:], in_=sq[:].rearrange("p (k b) -> p b k", k=N // P),
                                    axis=AX.X, op=op.add)
            ones_col = pool.tile([P, 1], f32)
            nc.vector.memset(ones_col[:], 1.0)
            psE = pp.tile([1, P], f32, space="PSUM", tag="psC0")
            nc.tensor.matmul(out=psE[:], lhsT=ones_col[:], rhs=part[:], start=True, stop=True)
            Eng = pool.tile([1, P], f32)
            nc.vector.tensor_copy(Eng[:], psE[:])
            s1 = pool.tile([1, P], f32)
            nc.scalar.activation(out=s1[:], in_=Eng[:], func=AF.Sqrt, bias=1.0, scale=1.0)
            dn = pool.tile([1, P], f32)
            nc.vector.tensor_scalar(out=dn[:], in0=s1[:], scalar1=1.0, scalar2=2.0,
                                    op0=op.add, op1=op.mult)
            rdn = pool.tile([1, P], f32)
            nc.vector.reciprocal(out=rdn[:], in_=dn[:])
            qq = pool.tile([1, P], f32)
            nc.vector.scalar_tensor_tensor(out=qq[:], in0=Eng[:], scalar=1.0, in1=rdn[:],
                                           op0=op.mult, op1=op.mult)
            lnq = pool.tile([1, P], f32)
            nc.scalar.activation(out=lnq[:], in_=qq[:], func=AF.Ln)

            rows = [pool.tile([1, P], f32, name=f"rows{s}") for s in range(3)]
            psH = pp.tile([20, P], f32, space="PSUM", tag="psC1")
            psO = pp.tile([1, P], f32, space="PSUM", tag="psC2")
            hX = pool.tile([20, P], f32)
            for s in range(3):
                nc.tensor.matmul(out=psH[:], lhsT=w1t[:, 20 * s:20 * (s + 1)], rhs=lnq[:],
                                 start=True, stop=True)
                nc.scalar.activation(out=hX[:], in_=psH[:], func=AF.Relu, bias=b1t[:, s:s + 1])
                nc.tensor.matmul(out=psO[:], lhsT=w2t[:, s:s + 1], rhs=hX[:], start=True, stop=True)
                nc.scalar.activation(out=rows[s][:], in_=psO[:], func=AF.Sigmoid,
                                     bias=b2t[:, s:s + 1])
            # transpose rows -> per-partition columns
            ones11 = pool.tile([1, 1], f32)
            nc.vector.memset(ones11[:], 1.0)
            psPP = pp.tile([P, 3], f32, space="PSUM", tag="psT")
            gpp = pool.tile([P, 1], f32)
            wipp = pool.tile([P, 1], f32)
            wepp = pool.tile([P, 1], f32)
            for s, dst in enumerate([gpp, wipp, wepp]):
                nc.tensor.matmul(out=psPP[:, s:s + 1], lhsT=rows[s][:], rhs=ones11[:],
                                 start=True, stop=True)
                nc.vector.tensor_copy(dst[:], psPP[:, s:s + 1])
            ompp = pool.tile([P, 1], f32)
            nc.vector.tensor_scalar(out=ompp[:], in0=gpp[:], scalar1=-1.0, scalar2=1.0,
                                    op0=op.mult, op1=op.add)
            gwepp = pool.tile([P, 1], f32)
            nc.vector.tensor_tensor(gwepp[:], gpp[:], wepp[:], op.mult)
            gwenpp = pool.tile([P, 1], f32)
            nc.vector.tensor_scalar_mul(out=gwenpp[:], in0=gwepp[:], scalar1=-1.0)
            gnpp = pool.tile([P, 1], f32)
            nc.vector.tensor_scalar_mul(out=gnpp[:], in0=gpp[:], scalar1=-0.125)
            # diag matrices
            nc.vector.tensor_scalar_mul(out=dOm[:], in0=ident[:], scalar1=ompp[:])
            nc.vector.tensor_scalar_mul(out=dG[:], in0=ident[:], scalar1=gpp[:])
            nc.vector.tensor_scalar_mul(out=dGn[:], in0=ident[:], scalar1=gnpp[:])
            nc.vector.tensor_scalar_mul(out=dGWe[:], in0=ident[:], scalar1=gwepp[:])
            nc.vector.tensor_scalar_mul(out=dGWen[:], in0=ident[:], scalar1=gwenpp[:])
            nc.vector.tensor_copy(dOm16[:], dOm[:])
            nc.vector.tensor_copy(dGn16[:], dGn[:])
            # ell = Wi * chn_mine^T
            for k in range(4):
                nc.tensor.transpose(out=psT[:, :P], in_=chm_nat[:, k * P:(k + 1) * P],
                                    identity=ident[:])
                nc.vector.tensor_scalar_mul(out=ell[:, k * P:(k + 1) * P], in0=psT[:, :P],
                                            scalar1=wipp[:])
            # init state
            nc.vector.memset(V[:], 0.0)
            nc.vector.memset(Ccol[:], 0.0)
            nc.vector.memset(Crow[:], 0.0)

            # --- helper emitters
            def emit_t(i):
                t = tts[i % 2]
                nc.vector.tensor_reduce(out=s3t[:], in_=Ccol[:].rearrange("p (v j) -> p v j", j=3),
                                        axis=AX.X, op=op.add)
                nc.vector.scalar_tensor_tensor(out=t[:], in0=s3t[:], scalar=0.125, in1=ell[:], op0=op.mult, op1=op.add)
                return t

            def emit_out(i, t):
                tout = touts[i % 2]
                for k in range(4):
                    nc.tensor.transpose(out=psT[:, k * P:(k + 1) * P],
                                        in_=t[:, k * P:(k + 1) * P], identity=ident[:])
                nc.vector.tensor_copy(tout[:], psT[:])
                nc.scalar.dma_start(
                    out=out_ext[i].rearrange("(k nl) b -> nl k b", k=4),
                    in_=tout[:].rearrange("p (k b) -> p k b", k=4))

            def transport(src16, sidx, agin, agout, rsh, rso, ridx, dst16):
                nc.gpsimd.local_scatter(out_ap=sbA[:], data_ap=src16[:], idxs_ap=sidx[:],
                                        channels=P, num_elems=SP, num_idxs=EL)
                nc.scalar.dma_start(out=agin[:], in_=sbA[:])
                nc.gpsimd.collective_compute(
                    "AllGather", op.bypass, replica_groups=[list(range(NC))],
                    ins=[agin[:]], outs=[agout[:]])
                # static block transpose: rsh[d, p, c*PAD:...] = agout[c, p, d*PAD:...]
                for c_ in range(NC):
                    nc.scalar.dma_start(
                        out=rsh[:, :, c_ * PAD:(c_ + 1) * PAD],
                        in_=agout[c_].rearrange("p (d f) -> d p f", d=NC))
                nc.gpsimd.collective_compute(
                    "ReduceScatter", op.add, replica_groups=[list(range(NC))],
                    ins=[rsh[:]], outs=[rso[:]])
                nc.scalar.dma_start(out=sbB[:], in_=rso[:])
                nc.gpsimd.local_scatter(out_ap=dst16[:], data_ap=sbB[:], idxs_ap=ridx[:],
                                        channels=P, num_elems=EL, num_idxs=SP)

            # --- main loop (fully unrolled)
            for i in range(T_run):
                t = emit_t(i)
                if i >= 1:
                    emit_out(i - 1, t)
                # V-damp: V' = (1-g)V + g*t[rep3] - g*Ccol ; lam = clip(V')
                nc.vector.tensor_copy(
                    t3t[:].rearrange("p (v j) -> p v j", j=3),
                    t[:].unsqueeze(2).broadcast_to([P, NL, 3]))
                for j in range(3):
                    sl = slice(512 * j, 512 * (j + 1))
                    nc.tensor.matmul(out=psV[j][:], lhsT=dOm[:], rhs=V[:, sl],
                                     start=True, stop=False)
                    nc.tensor.matmul(out=psV[j][:], lhsT=dG[:], rhs=t3t[:, sl],
                                     start=False, stop=False)
                    nc.tensor.matmul(out=psV[j][:], lhsT=dGn16[:], rhs=Ccol[:, sl],
                                     start=False, stop=True)
                    nc.vector.tensor_scalar(out=lam16[:, sl], in0=psV[j][:],
                                            scalar1=LLR_CLIP, scalar2=-LLR_CLIP,
                                            op0=op.min, op1=op.max)
                    nc.vector.tensor_copy(V[:, sl], psV[j][:])
                transport(lam16, issc, aginL, agoutL, rshL, rsoL, irsc, lamR)
                # row compute
                nc.scalar.activation(out=u0t[:], in_=lamR[:], func=AF.Tanh, scale=0.5 / 8.0)
                nc.vector.tensor_scalar(out=ut[:], in0=u0t[:], scalar1=UCLIP,
                                        scalar2=-UCLIP, op0=op.min, op1=op.max)
                nc.vector.tensor_reduce(out=Prt[:], in_=ut[:].rearrange("p (m k) -> p m k", k=6),
                                        axis=AX.X, op=op.mult)
                nc.vector.reciprocal(out=rct[:], in_=ut[:])
                nc.vector.tensor_tensor(
                    wct[:].rearrange("p (m k) -> p m k", k=6),
                    Prt[:].unsqueeze(2).broadcast_to([P, ML, 6]),
                    rct[:].rearrange("p (m k) -> p m k", k=6), op.mult)
                nc.vector.tensor_scalar(out=wct[:], in0=wct[:], scalar1=EPS1,
                                        scalar2=-EPS1, op0=op.min, op1=op.max)
                nc.scalar.activation(out=t1t[:], in_=wct[:], func=AF.Ln, bias=1.0, scale=EPS1)
                nc.scalar.activation(out=t2t[:], in_=wct[:], func=AF.Ln, bias=1.0, scale=-EPS1)
                # C-damp: Crow' = (1-g)Crow + gWe*t1 - gWe*t2
                for j in range(3):
                    sl = slice(512 * j, 512 * (j + 1))
                    nc.tensor.matmul(out=psC[j][:], lhsT=dOm16[:], rhs=Crow[:, sl],
                                     start=True, stop=False)
                    nc.tensor.matmul(out=psC[j][:], lhsT=dGWe[:], rhs=t1t[:, sl],
                                     start=False, stop=False)
                    nc.tensor.matmul(out=psC[j][:], lhsT=dGWen[:], rhs=t2t[:, sl],
                                     start=False, stop=True)
                    nc.vector.tensor_copy(Crow[:, sl], psC[j][:])
                transport(Crow, issr, aginR, agoutR, rshR, rsoR, irsr, Ccol)
            # final output
            t = emit_t(T_run)
            emit_out(T_run - 1, t)

    nc.finalize()
    return nc


def kernel(**inputs):
    chn = np.ascontiguousarray(np.asarray(inputs["chn_llr"], np.float32))
    row_idx = np.asarray(inputs["row_idx"])
    col_idx = np.asarray(inputs["col_idx"])
    key = (row_idx.tobytes(), col_idx.tobytes())
    if "tables" not in _cache or _cache.get("key") != key:
        _cache["tables"] = _build_tables(row_idx, col_idx)
        _cache["key"] = key
        _cache.pop("nc", None)
    tb = _cache["tables"]
    PAD = tb["PAD"]
    T_run = int(_cache.get("T_run", T))
    if "nc" not in _cache:
        _cache["nc"] = _build_program(PAD, T_run)
    nc = _cache["nc"]

    w1s = np.stack([np.asarray(inputs[k], np.float32).reshape(20, 1).T * LOG10SCALE
                    for k in ("gW1", "iW1", "eW1")]).astype(np.float32)      # (3,1,20)
    b1s = np.stack([np.asarray(inputs[k], np.float32).reshape(20, 1)
                    for k in ("gb1", "ib1", "eb1")]).astype(np.float32)      # (3,20,1)
    w2s = np.stack([np.asarray(inputs[k], np.float32).reshape(1, 20).T
                    for k in ("gW2", "iW2", "eW2")]).astype(np.float32)      # (3,20,1)
    b2s = np.stack([np.asarray(inputs[k], np.float32).reshape(1, 1)
                    for k in ("gb2", "ib2", "eb2")]).astype(np.float32)      # (3,1,1)
    idn = np.eye(P, dtype=np.float32)

    in_maps = []
    for c in range(NC):
        in_maps.append({
            "chn": chn,
            "chm": np.ascontiguousarray(chn[c * NL:(c + 1) * NL, :]),
            "idn": idn,
            "w1s": w1s, "b1s": b1s, "w2s": w2s, "b2s": b2s,
            "ssc": np.ascontiguousarray(np.tile(tb["ssc"][c][None, :], (P, 1))),
            "rsc": np.ascontiguousarray(np.tile(tb["rsc"][c][None, :], (P, 1))),
            "ssr": np.ascontiguousarray(np.tile(tb["ssr"][c][None, :], (P, 1))),
            "rsr": np.ascontiguousarray(np.tile(tb["rsr"][c][None, :], (P, 1))),
        })
    if "sharded" not in _cache:
        _cache["sharded"] = _make_sharded(nc)
    sharded, in_names, out_names, out_avals, n_params = _cache["sharded"]
    import jax
    import jax.numpy as jnp
    from jax.sharding import NamedSharding, PartitionSpec
    mesh = _cache["mesh"]
    shd = NamedSharding(mesh, PartitionSpec("core"))
    import hashlib
    h = hashlib.sha1()
    for nm in ("chn_llr", "gW1", "gW2", "iW1", "iW2", "eW1", "eW2", "gb1", "ib1", "eb1"):
        h.update(np.ascontiguousarray(np.asarray(inputs[nm])).tobytes())
    ikey = h.hexdigest()
    if _cache.get("dev_key") != ikey:
        per_core = [[np.asarray(in_maps[c][nm]) for nm in in_names[:n_params]]
                    for c in range(NC)]
        concat_in = [np.concatenate([per_core[c][i] for c in range(NC)], axis=0)
                     for i in range(n_params)]
        _cache["dev_in"] = [jax.device_put(a, shd) for a in concat_in]
        _cache["dev_key"] = ikey
    if "dev_zeros" not in _cache:
        _cache["dev_zeros"] = [jnp.zeros((NC * a.shape[0], *a.shape[1:]), a.dtype,
                                         device=shd) for a in out_avals]
    out_arrs = sharded(*_cache["dev_in"], *_cache["dev_zeros"])
    oi = out_names.index("out")
    arr = out_arrs[oi]
    T_r = out_avals[oi].shape[0]
    out = np.empty((T_r, N, B), np.float32)
    from concurrent.futures import ThreadPoolExecutor

    def fetch(s):
        c = s.index[0].start // T_r if s.index[0].start else 0
        out[:, c * NL:(c + 1) * NL, :] = np.asarray(s.data)
        return None

    with ThreadPoolExecutor(NC) as ex:
        list(ex.map(fetch, arr.addressable_shards))
    return out


def _make_sharded(nc):
    import jax
    import jax.numpy as jnp
    from jax.experimental.shard_map import shard_map
    from jax.sharding import Mesh, PartitionSpec
    import concourse.mybir as mybir
    from concourse import bass2jax
    from concourse.bass2jax import _bass_exec_p, partition_id_tensor, install_neuronx_cc_hook

    install_neuronx_cc_hook()
    partition_name = nc.partition_id_tensor.name if nc.partition_id_tensor else None
    in_names, out_names, out_avals = [], [], []
    for alloc in nc.m.functions[0].allocations:
        if not isinstance(alloc, mybir.MemoryLocationSet):
            continue
        name = alloc.memorylocations[0].name
        if alloc.kind == "ExternalInput":
            if name != partition_name:
                in_names.append(name)
        elif alloc.kind == "ExternalOutput":
            shape = tuple(alloc.tensor_shape)
            out_avals.append(jax.core.ShapedArray(shape, mybir.dt.np(alloc.dtype)))
            out_names.append(name)
    n_params = len(in_names)
    n_outs = len(out_avals)
    in_names = in_names + out_names
    if partition_name is not None:
        in_names.append(partition_name)
    donate = ()

    def _body(*args):
        operands = list(args)
        if partition_name is not None:
            operands.append(partition_id_tensor())
        outs = _bass_exec_p.bind(
            *operands,
            out_avals=tuple(out_avals),
            in_names=tuple(in_names),
            out_names=tuple(out_names),
            lowering_input_output_aliases=(),
            sim_require_finite=True,
            sim_require_nnan=True,
            nc=nc,
        )
        return tuple(outs)

    devices = jax.devices()[:NC]
    mesh = Mesh(np.asarray(devices), ("core",))
    _cache["mesh"] = mesh
    sharded = jax.jit(
        shard_map(_body, mesh=mesh,
                  in_specs=(PartitionSpec("core"),) * (n_params + n_outs),
                  out_specs=(PartitionSpec("core"),) * n_outs,
                  check_rep=False),
        donate_argnums=donate, keep_unused=True)
    return sharded, in_names, out_names, out_avals, n_params
